# revision 10
# baseline (speedup 1.0000x reference)
"""GAT (2-layer, PyG-default) Trainium2 Bass kernel, 8-core SPMD.

v3 — trace-driven rework of the dst-major design:
  - Node permutation is globally degree-balanced: nodes ranked by
    in-degree are dealt into (chunk, core, lane) so each chunk's 1024
    lanes (128 per core x 8 cores) have near-uniform degree, shrinking
    the per-chunk slot count K toward the mean degree.
  - L1 rows are stored feature-transposed (c,h): the per-edge softmax
    weight broadcast then has unit inner stride on every operand, so
    the one big per-chunk message multiply runs in DVE 2x_1P mode
    (in-place on the gather tile; no per-slot DVE ops).
  - L2 gathers PAIRS of compact 128B tb2 rows (idx = src>>1, 256B
    descriptors): half the SWDGE descriptors of v2, and the pair index
    range (25088 < 32768) fits one int16 window -> exactly one gather
    call per chunk.  Even/odd row selection is folded into the alpha
    masks (me/mo).  L2 self-loop contributions come from an
    SBUF-resident tb2self captured while L1 writes tb2 rows, so they
    never touch the gather path.
  - Phase 0 uses a single 528-col matmul per 128-row group and splits
    the PSUM->bf16 casts between Vector and Scalar; transposes and the
    W2 projection run in bf16.

Self-contained: only needs numpy + the concourse tree at /opt/trn_rl_repo.
"""

import hashlib
import sys

import numpy as np

for _p in ("/opt/trn_rl_repo",):
    if _p not in sys.path:
        sys.path.insert(0, _p)

import concourse.bacc as bacc
import concourse.bass as bass
import concourse.tile as tile
from concourse import mybir
from concourse.bass_utils import run_bass_kernel_spmd

F32 = mybir.dt.float32
BF16 = mybir.dt.bfloat16
I16 = mybir.dt.int16
AF = mybir.ActivationFunctionType
OP = mybir.AluOpType
AX = mybir.AxisListType

N_CORES = 8
SPAN = 32768
W = 4
GMAX = 8
_QCTR = [0]  # global SWDGE queue round-robin


# ----------------------------------------------------------------------------
# Host-side edge planning
# ----------------------------------------------------------------------------

def _wrap_idx(si, n_cores, K):
    """[n_cores,128,K] int -> [n_cores,16,8K] in the dma_gather idx layout
    (idx of token T, partition p lands at [p%16, 8*T + p//16])."""
    tmp = si.reshape(n_cores, 8, 16, K)
    return np.ascontiguousarray(tmp.transpose(0, 2, 3, 1)).reshape(
        n_cores, 16, 8 * K)


def _edge_plan(edge_index, N, n_cores, nch, npcp):
    """Degree-balanced dst-major plans for both layers.

    L1: self-loops included, W=4 overlapping 32768-row windows.
    L2: self-loops excluded, rows gathered in PAIRS (idx = srow>>1) from
        a single window, with even/odd masks me/mo.
    """
    R = n_cores * npcp
    bases = [round(q * (R - SPAN) / (W - 1)) for q in range(W)]

    src1 = np.concatenate([np.asarray(edge_index[0], np.int64), np.arange(N)])
    dst1 = np.concatenate([np.asarray(edge_index[1], np.int64), np.arange(N)])
    deg = np.bincount(dst1, minlength=R)
    # Deal degree-ranked nodes into (chunk, core, lane): chunk lanes get
    # near-uniform degree across all cores.
    order = np.argsort(deg, kind="stable")
    blk = 128 * n_cores
    ii = np.arange(R)
    chunk = ii // blk
    core = (ii % blk) // 128
    lane = ii % 128
    pos = np.empty(R, np.int64)
    pos[order] = core * npcp + chunk * 128 + lane

    def build_emat(src, dst):
        srow = pos[src]
        dpos = pos[dst]
        key = dpos // npcp * (nch * 128) + dpos % npcp
        order_e = np.lexsort((srow, key))
        ks, ss = key[order_e], srow[order_e]
        counts = np.bincount(ks, minlength=n_cores * nch * 128)
        maxd = max(int(counts.max()), 1)
        starts = np.zeros(len(counts) + 1, np.int64)
        np.cumsum(counts, out=starts[1:])
        col = np.arange(len(ss)) - starts[ks]
        E = np.full((n_cores * nch * 128, maxd), 2**31, np.int64)
        E[ks, col] = ss
        return E, counts

    Emat1, counts1 = build_emat(src1, dst1)
    Emat2, counts2 = build_emat(np.asarray(edge_index[0], np.int64),
                                np.asarray(edge_index[1], np.int64))

    def plan_chunk(E, degv):
        valid = E < 2**31
        A = np.zeros(W, np.int64)
        B = np.zeros(W, np.int64)
        dmax = int(degv.max())
        for q in range(W - 1):
            A[q] = int(((E < bases[q + 1]) & valid).sum(axis=1).max())
            B[q] = int(((E >= bases[q] + SPAN) & valid).sum(axis=1).max())
        A[W - 1] = dmax
        K = int(max(dmax, (A + B).max(), 1))
        L = E.shape[0]
        maxd = E.shape[1]
        while True:
            P = np.maximum.accumulate(np.minimum(np.maximum(A, 0), K - B))
            P[W - 1] = K
            n = np.diff(np.concatenate([[0], P]))
            qcls = np.repeat(np.arange(W), n)
            ptr = np.zeros(L, np.int64)
            slotidx = np.zeros((L, K), np.int32)
            slotmask = np.zeros((L, K), bool)
            ok = True
            for t in range(K):
                b = bases[qcls[t]]
                cur = E[np.arange(L), np.minimum(ptr, maxd - 1)]
                vv = ptr < degv
                if np.any(vv & (cur < b)):
                    ok = False
                    break
                fit = vv & (cur >= b) & (cur < b + SPAN)
                slotidx[:, t] = np.where(fit, cur - b, 0)
                slotmask[:, t] = fit
                ptr += fit
            if ok and np.all(ptr == degv):
                return K, qcls, slotidx, slotmask
            K += 1
            assert K < dmax + 24, "edge window planning failed to converge"

    Ks, toff, calls = [], [], []
    blocks_idx, blocks_mask = [], []
    K2s, toff2 = [], []
    blocks_idx2, blocks_me, blocks_mo = [], [], []
    off = 0
    off2 = 0
    for c in range(nch):
        lanes = ((np.arange(n_cores)[:, None] * nch + c) * 128
                 + np.arange(128)[None, :]).ravel()
        # ---- L1 (windowed, self-loops in-plan) ----
        K, qcls, si, sm = plan_chunk(Emat1[lanes], counts1[lanes])
        Ks.append(K)
        toff.append(off)
        cc = []
        t0 = 0
        while t0 < K:
            q = qcls[t0]
            t1 = t0
            while t1 < K and qcls[t1] == q and t1 - t0 < GMAX:
                t1 += 1
            cc.append((t0, t1, int(q)))
            t0 = t1
        calls.append(cc)
        blocks_idx.append(_wrap_idx(si.reshape(n_cores, 128, K), n_cores, K))
        blocks_mask.append(sm.reshape(n_cores, 128, K))
        off += K
        # ---- L2 (paired rows, single window, no self-loops) ----
        E2 = Emat2[lanes]
        degv2 = counts2[lanes]
        K2 = max(int(degv2.max()), 1)
        sub = E2[:, :K2]
        valid = np.arange(K2)[None, :] < degv2[:, None]
        idx2 = np.where(valid, sub >> 1, 0).astype(np.int32)
        par = np.where(valid, sub & 1, 0)
        me = (valid & (par == 0)).astype(np.float32)
        mo = (valid & (par == 1)).astype(np.float32)
        K2s.append(K2)
        toff2.append(off2)
        blocks_idx2.append(_wrap_idx(idx2.reshape(n_cores, 128, K2),
                                     n_cores, K2))
        blocks_me.append(me.reshape(n_cores, 128, K2))
        blocks_mo.append(mo.reshape(n_cores, 128, K2))
        off2 += K2
    TOT, TOT2 = off, off2
    idx16 = np.tile(np.concatenate(blocks_idx, axis=2).astype(np.int16),
                    (1, 8, 1))
    mask = np.concatenate(blocks_mask, axis=2).astype(np.float32)
    idx16_2 = np.tile(np.concatenate(blocks_idx2, axis=2).astype(np.int16),
                      (1, 8, 1))
    me_all = np.concatenate(blocks_me, axis=2)
    mo_all = np.concatenate(blocks_mo, axis=2)
    return (pos, bases, Ks, toff, TOT, calls, idx16, mask,
            K2s, toff2, TOT2, idx16_2, me_all, mo_all)


def _host_prep(x, edge_index, W1, att1_src, att1_dst, W2, att2_src, att2_dst):
    N, F = x.shape
    H, C = att1_src.shape
    HC = H * C
    NCLS = W2.shape[1]
    n_cores = N_CORES
    nch = -(-N // (n_cores * 128))
    npcp = nch * 128
    R = n_cores * npcp

    (pos, bases, Ks, toff, TOT, calls, idx16, mask,
     K2s, toff2, TOT2, idx16_2, me_all, mo_all) = _edge_plan(
        edge_index, N, n_cores, nch, npcp)

    # (c,h) feature permutation: new col c*H+h <- old col h*C+c
    jj = np.arange(HC)
    perm = (jj % H) * C + jj // H
    W1p = W1[:, perm]

    # Folded attention-logit weight columns (independent of column order)
    Wa_s = np.einsum("fhc,hc->fh", W1.reshape(F, H, C), att1_src).astype(np.float32)
    Wa_d = np.einsum("fhc,hc->fh", W1.reshape(F, H, C), att1_dst).astype(np.float32)
    W1e = np.ascontiguousarray(
        np.concatenate([W1p, Wa_s, Wa_d], axis=1), dtype=np.float32)  # [F, 528]

    w2s = (W2 @ att2_src[0]).astype(np.float32)
    w2d = (W2 @ att2_dst[0]).astype(np.float32)
    W2e_flat = np.zeros((HC, 64), np.float32)
    W2e_flat[:, :NCLS] = W2[perm]
    W2e_flat[:, NCLS] = w2s[perm]
    W2e_flat[:, NCLS + 1] = w2d[perm]
    nslab = HC // 128
    W2e = np.ascontiguousarray(
        W2e_flat.reshape(nslab, 128, 64).transpose(1, 0, 2))  # [128, 4, 64]

    import ml_dtypes
    bf = ml_dtypes.bfloat16
    xtab = np.zeros((R, F), np.float32)
    xtab[pos[np.arange(N)]] = x
    xTp = np.ascontiguousarray(xtab.T).astype(bf)   # [F, R] permuted cols
    W1e = W1e.astype(bf)
    ident = np.eye(128, dtype=np.float32).astype(bf)

    cfg = dict(
        N=N, F=F, H=H, C=C, HC=HC, NCLS=NCLS, n_cores=n_cores,
        nch=nch, npcp=npcp, R=R, nslab=nslab,
        Ks=Ks, toff=toff, TOT=TOT, calls=calls, bases=bases, pos=pos,
        K2s=K2s, toff2=toff2, TOT2=TOT2,
        swdge_queues=4, p0_bufs=4,
        gt_bufs=3 if max(Ks) <= 30 else 2, g2_bufs=3,
    )
    shared = dict(xTp=xTp, W1e=W1e, W2e=W2e.astype(bf), ident=ident)
    per_core = [
        dict(g1idx=idx16[k], mask=mask[k].astype(bf),
             g2idx=idx16_2[k], me=me_all[k], mo=mo_all[k])
        for k in range(n_cores)
    ]
    return cfg, shared, per_core


# ----------------------------------------------------------------------------
# Device program
# ----------------------------------------------------------------------------

def _build_program(cfg):
    F, HC, NCLS = cfg["F"], cfg["HC"], cfg["NCLS"]
    n_cores, npcp, R = cfg["n_cores"], cfg["npcp"], cfg["R"]
    nslab, TOT, TOT2 = cfg["nslab"], cfg["TOT"], cfg["TOT2"]

    nc = bacc.Bacc("TRN2", target_bir_lowering=False, debug=False,
                   num_devices=n_cores,
                   num_swdge_queues=cfg.get("swdge_queues", 1))

    xTp = nc.dram_tensor("xTp", [F, R], BF16, kind="ExternalInput").ap()
    W1e = nc.dram_tensor("W1e", [F, HC + 16], BF16, kind="ExternalInput").ap()
    W2e = nc.dram_tensor("W2e", [128, nslab, 64], BF16, kind="ExternalInput").ap()
    ident_d = nc.dram_tensor("ident", [128, 128], BF16, kind="ExternalInput").ap()
    g1idx = nc.dram_tensor("g1idx", [128, 8 * TOT], I16,
                           kind="ExternalInput").ap()
    mask_d = nc.dram_tensor("mask", [128, TOT], BF16, kind="ExternalInput").ap()
    g2idx = nc.dram_tensor("g2idx", [128, 8 * TOT2], I16,
                           kind="ExternalInput").ap()
    me_d = nc.dram_tensor("me", [128, TOT2], F32, kind="ExternalInput").ap()
    mo_d = nc.dram_tensor("mo", [128, TOT2], F32, kind="ExternalInput").ap()

    T1 = nc.dram_tensor("T1", [R, 640], BF16).ap()
    tb2_own = nc.dram_tensor("tb2_own", [npcp, 64], BF16).ap()
    tb2_full = nc.dram_tensor("tb2_full", [R, 64], BF16,
                              addr_space="Shared").ap()
    out2 = nc.dram_tensor("out2", [npcp, NCLS], F32, kind="ExternalOutput").ap()

    tensors = dict(xTp=xTp, W1e=W1e, W2e=W2e, ident=ident_d, g1idx=g1idx,
                   mask=mask_d, g2idx=g2idx, me=me_d, mo=mo_d,
                   T1=T1, tb2_own=tb2_own, tb2_full=tb2_full, out2=out2)
    repeat = cfg.get("repeat", 1)
    with tile.TileContext(nc) as tc:
        for _ in range(repeat):
            _emit(tc, cfg, tensors)
    nc.compile()
    return nc


def _emit(tc, cfg, t):
    nc = tc.nc
    H, HC, NCLS = cfg["H"], cfg["HC"], cfg["NCLS"]
    n_cores, nch, npcp, R = cfg["n_cores"], cfg["nch"], cfg["npcp"], cfg["R"]
    nslab = cfg["nslab"]
    NTB = R // 128

    with tc.tile_pool(name="consts", bufs=1) as cpool:
        W1e_sb = cpool.tile([128, HC + 16], BF16)
        nc.sync.dma_start(W1e_sb[:], t["W1e"][:, :])
        W2e_sb = cpool.tile([128, nslab, 64], BF16)
        nc.sync.dma_start(W2e_sb[:], t["W2e"][:, :, :])
        ident_bf = cpool.tile([128, 128], BF16)
        nc.sync.dma_start(ident_bf[:], t["ident"][:, :])
        ald1_all = cpool.tile([128, NTB, H], F32)
        ald1_sb = cpool.tile([128, nch, H], F32)
        tb2self = cpool.tile([128, nch, 64], BF16)

        # ---------------- Phase 0: permuted node table T1 ----------------
        with (
            nc.named_scope("p0"),
            tc.tile_pool(name="p0", bufs=cfg.get("p0_bufs", 4)) as pool,
            tc.tile_pool(name="p0ps", bufs=cfg.get("p0_bufs", 4),
                         space="PSUM") as pps,
        ):
            nblk = R // 512
            for i in range(nblk):
                xt = pool.tile([128, 512], BF16, tag="xt")
                nc.sync.dma_start(xt[:], t["xTp"][:, 512 * i: 512 * i + 512])
                rowB = pool.tile([128, 4, HC + 16], BF16, tag="rowB")
                for j in range(4):
                    # two matmuls: a 512-col PSUM write must stay in one bank
                    ps = pps.tile([128, HC + 16], F32, tag="ps")
                    nc.tensor.matmul(ps[:, 0:HC],
                                     lhsT=xt[:, 128 * j: 128 * j + 128],
                                     rhs=W1e_sb[:, 0:HC], start=True, stop=True)
                    nc.tensor.matmul(ps[:, HC: HC + 16],
                                     lhsT=xt[:, 128 * j: 128 * j + 128],
                                     rhs=W1e_sb[:, HC: HC + 16],
                                     start=True, stop=True)
                    if j % 2 == 0:
                        nc.vector.tensor_copy(rowB[:, j, 0:HC], ps[:, 0:HC])
                    else:
                        nc.scalar.copy(rowB[:, j, 0:HC], ps[:, 0:HC])
                    nc.vector.tensor_copy(
                        rowB[:, j, HC: HC + 16].bitcast(F32),
                        ps[:, HC: HC + H])
                    nc.scalar.copy(ald1_all[:, 4 * i + j, :],
                                   ps[:, HC + H: HC + 2 * H])
                # ACT HWDGE ring: don't serialize behind xt reads on SP ring
                nc.scalar.dma_start(
                    t["T1"][512 * i: 512 * i + 512, 0: HC + 16].rearrange(
                        "(j p) c -> p j c", p=128),
                    rowB[:],
                )

        pid = nc.partition_id()
        nc.sync.dma_start(ald1_sb[:], ald1_all[:, bass.ds(pid * nch, nch), :])

        if cfg.get("phases", "full") == "p0":
            return
        # ---------------- L1 edge phase ----------------
        with nc.named_scope("l1"):
            _l1_phase(tc, cfg, t, W2e_sb, ident_bf, ald1_sb, tb2self)

        if cfg.get("phases", "full") == "p0+l1":
            return
        # ---------------- allgather ----------------
        with nc.named_scope("ag"):
            if cfg.get("no_collective"):
                nc.sync.dma_start(t["tb2_full"][0:npcp, :], t["tb2_own"][:, :])
            else:
                nc.gpsimd.collective_compute(
                    "AllGather",
                    OP.bypass,
                    replica_groups=[list(range(n_cores))],
                    ins=[t["tb2_own"][:, :]],
                    outs=[t["tb2_full"][:, :]],
                )

        if cfg.get("phases", "full") == "p0+l1+ag":
            return
        # ---------------- L2 edge phase ----------------
        with nc.named_scope("l2"):
            _l2_phase(tc, cfg, t, tb2self)


def _l1_phase(tc, cfg, t, W2e_sb, ident_bf, ald1_sb, tb2self):
    nc = tc.nc
    nch, H, C, HC, NCLS = cfg["nch"], cfg["H"], cfg["C"], cfg["HC"], cfg["NCLS"]
    nslab = cfg["nslab"]
    Ks, toff, calls, bases = cfg["Ks"], cfg["toff"], cfg["calls"], cfg["bases"]
    TOT = cfg["TOT"]
    nq = cfg.get("swdge_queues", 1)

    with (
        tc.tile_pool(name="gt1", bufs=cfg.get("gt_bufs", 2)) as gpool,
        tc.tile_pool(name="meta1", bufs=1) as mpool,
        tc.tile_pool(name="small1", bufs=3) as smpool,
        tc.tile_pool(name="out1", bufs=2) as opool,
        tc.tile_pool(name="ps_u1", bufs=2, space="PSUM") as pp_u,
        tc.tile_pool(name="ps_tr1", bufs=2, space="PSUM") as pp_tr,
        tc.tile_pool(name="ps_o1", bufs=2, space="PSUM") as pp_o,
    ):
        idx_all = mpool.tile([128, 8 * TOT], I16, tag="idxall")
        nc.sync.dma_start(idx_all[:], t["g1idx"][:, :])
        msk_all = mpool.tile([128, TOT], BF16, tag="mskall")
        nc.sync.dma_start(msk_all[:], t["mask"][:, :])
        for c in range(nch):
            K = Ks[c]
            off = toff[c]
            gt = gpool.tile([128, K, 640], BF16, tag="gt")
            for (b0, b1, q) in calls[c]:
                nk = b1 - b0
                nc.gpsimd.dma_gather(
                    gt[:, b0:b1, :],
                    t["T1"][bases[q]: bases[q] + SPAN, :],
                    idx_all[:, 8 * (off + b0): 8 * (off + b1)],
                    nk * 128, nk * 128, 640,
                    queue_num=_QCTR[0] % nq,
                )
                _QCTR[0] += 1
            stop = cfg.get("l1_stop")
            if stop == "gather":
                continue

            # p = exp(leakyrelu(al_src[src] + al_dst[dst])) * mask
            s_t = smpool.tile([128, K, H], F32, tag="s")
            nc.vector.tensor_tensor(
                s_t[:],
                gt[:, :, HC: HC + 16].bitcast(F32),
                ald1_sb[:, c, None, :].to_broadcast([128, K, H]),
                op=OP.add,
            )
            l_t = smpool.tile([128, K, H], F32, tag="l")
            nc.vector.scalar_tensor_tensor(
                l_t[:], s_t[:], 0.2, s_t[:], op0=OP.mult, op1=OP.max
            )
            p_t = smpool.tile([128, K, H], F32, tag="p")
            nc.scalar.activation(p_t[:], l_t[:], AF.Exp)
            p_bf = smpool.tile([128, K, H], BF16, tag="pbf")
            nc.vector.tensor_tensor(
                p_bf[:], p_t[:],
                msk_all[:, off: off + K, None].to_broadcast([128, K, H]),
                op=OP.mult,
            )
            zr = smpool.tile([128, H], F32, tag="zr")
            nc.vector.tensor_reduce(
                zr[:], p_bf[:].rearrange("p k h -> p h k"),
                axis=AX.X, op=OP.add,
            )
            zb = smpool.tile([128, H], F32, tag="zb")
            nc.vector.tensor_scalar_max(zb[:], zr[:], 1e-30)
            rz = smpool.tile([128, H], F32, tag="rz")
            nc.vector.reciprocal(rz[:], zb[:])
            if stop == "logits":
                continue

            # in-place alpha multiply, (c,h) layout -> unit inner stride
            nc.vector.tensor_tensor(
                gt[:, :, 0:HC].rearrange("p k (c h) -> p k c h", h=H),
                gt[:, :, 0:HC].rearrange("p k (c h) -> p k c h", h=H),
                p_bf[:, :, None, :].to_broadcast([128, K, C, H]),
                op=OP.mult,
            )
            if stop == "mult":
                continue
            ps_u = pp_u.tile([128, HC], F32, tag="u")
            for k in range(K):
                nc.tensor.matmul(
                    ps_u[:], lhsT=ident_bf[:], rhs=gt[:, k, 0:HC],
                    start=(k == 0), stop=(k == K - 1),
                )
            if stop == "agg":
                continue
            h2 = opool.tile([128, HC], F32, tag="h2")
            nc.vector.tensor_tensor(
                h2[:].rearrange("p (c h) -> p c h", h=H),
                ps_u[:].rearrange("p (c h) -> p c h", h=H),
                rz[:, None, :].to_broadcast([128, C, H]),
                op=OP.mult,
            )
            h2r = opool.tile([128, HC], BF16, tag="h2r")
            nc.scalar.activation(h2r[:], h2[:], AF.Relu)
            if stop == "h2":
                continue
            ps_o = pp_o.tile([128, 64], F32, tag="o")
            for j in range(nslab):
                ps_tr = pp_tr.tile([128, 128], BF16, tag="tr")
                nc.tensor.transpose(
                    ps_tr[:], h2r[:, 128 * j: 128 * (j + 1)], ident_bf[:]
                )
                h2t = smpool.tile([128, 128], BF16, tag="h2t")
                nc.scalar.copy(h2t[:], ps_tr[:])
                nc.tensor.matmul(
                    ps_o[:], lhsT=h2t[:], rhs=W2e_sb[:, j, :],
                    start=(j == 0), stop=(j == nslab - 1),
                )
            # tb2 row: [40 cls bf16 | al2_src f32 pair | al2_dst f32 pair | pad]
            nc.vector.tensor_copy(tb2self[:, c, 0:NCLS], ps_o[:, 0:NCLS])
            nc.scalar.copy(tb2self[:, c, NCLS: NCLS + 4].bitcast(F32),
                           ps_o[:, NCLS: NCLS + 2])
            nc.sync.dma_start(t["tb2_own"][128 * c: 128 * (c + 1), :],
                              tb2self[:, c, :])


def _l2_phase(tc, cfg, t, tb2self):
    nc = tc.nc
    nch, NCLS = cfg["nch"], cfg["NCLS"]
    K2s, toff2, TOT2 = cfg["K2s"], cfg["toff2"], cfg["TOT2"]
    nq = cfg.get("swdge_queues", 1)

    with (
        tc.tile_pool(name="gt2", bufs=cfg.get("g2_bufs", 3)) as gpool,
        tc.tile_pool(name="meta2", bufs=1) as mpool,
        tc.tile_pool(name="small2", bufs=3) as smpool,
        tc.tile_pool(name="out2p", bufs=2) as opool,
    ):
        idx_all = mpool.tile([128, 8 * TOT2], I16, tag="idx2all")
        nc.sync.dma_start(idx_all[:], t["g2idx"][:, :])
        me_all = mpool.tile([128, TOT2], F32, tag="meall")
        nc.sync.dma_start(me_all[:], t["me"][:, :])
        mo_all = mpool.tile([128, TOT2], F32, tag="moall")
        nc.sync.dma_start(mo_all[:], t["mo"][:, :])
        tb2p = t["tb2_full"].rearrange("(a b) c -> a (b c)", b=2)
        for c in range(nch):
            K = K2s[c]
            off = toff2[c]
            me = me_all[:, off: off + K]
            mo = mo_all[:, off: off + K]
            gt = gpool.tile([128, K, 128], BF16, tag="gt2")
            for b0 in range(0, K, GMAX):
                b1 = min(b0 + GMAX, K)
                nk = b1 - b0
                nc.gpsimd.dma_gather(
                    gt[:, b0:b1, :], tb2p[:, :],
                    idx_all[:, 8 * (off + b0): 8 * (off + b1)],
                    nk * 128, nk * 128, 128,
                    queue_num=_QCTR[0] % nq,
                )
                _QCTR[0] += 1
            # logits: select even/odd al2_src, add own al2_dst
            ald = tb2self[:, c, NCLS + 2: NCLS + 4].bitcast(F32)  # [128,1]
            v1 = smpool.tile([128, K], F32, tag="v1")
            nc.vector.tensor_tensor(
                v1[:], gt[:, :, NCLS: NCLS + 2].bitcast(F32)[:, :, 0], me,
                op=OP.mult)
            v2 = smpool.tile([128, K], F32, tag="v2")
            nc.vector.tensor_tensor(
                v2[:], gt[:, :, 64 + NCLS: 64 + NCLS + 2].bitcast(F32)[:, :, 0],
                mo, op=OP.mult)
            s2 = smpool.tile([128, K], F32, tag="s2")
            nc.vector.tensor_tensor(s2[:], v1[:], v2[:], op=OP.add)
            s2b = smpool.tile([128, K], F32, tag="s2b")
            nc.vector.tensor_tensor(
                s2b[:], s2[:], ald.to_broadcast([128, K]), op=OP.add)
            l2t = smpool.tile([128, K], F32, tag="l2")
            nc.vector.scalar_tensor_tensor(
                l2t[:], s2b[:], 0.2, s2b[:], op0=OP.mult, op1=OP.max)
            p2 = smpool.tile([128, K], F32, tag="p2")
            nc.scalar.activation(p2[:], l2t[:], AF.Exp)
            aE = smpool.tile([128, K], BF16, tag="aE")
            nc.vector.tensor_tensor(aE[:], p2[:], me, op=OP.mult)
            aO = smpool.tile([128, K], BF16, tag="aO")
            nc.vector.tensor_tensor(aO[:], p2[:], mo, op=OP.mult)
            vs = smpool.tile([128, K], F32, tag="vs")
            nc.vector.tensor_tensor(vs[:], me, mo, op=OP.add)
            pm = smpool.tile([128, K], F32, tag="pm")
            nc.vector.tensor_tensor(pm[:], p2[:], vs[:], op=OP.mult)
            z2 = smpool.tile([128, 1], F32, tag="z2")
            nc.vector.tensor_reduce(z2[:], pm[:], axis=AX.X, op=OP.add)
            # self-loop: logit from own al2_src + own al2_dst
            ss = smpool.tile([128, 1], F32, tag="ss")
            nc.vector.tensor_tensor(
                ss[:], tb2self[:, c, NCLS: NCLS + 2].bitcast(F32), ald,
                op=OP.add)
            lss = smpool.tile([128, 1], F32, tag="lss")
            nc.vector.scalar_tensor_tensor(
                lss[:], ss[:], 0.2, ss[:], op0=OP.mult, op1=OP.max)
            p2s = smpool.tile([128, 1], F32, tag="p2s")
            nc.scalar.activation(p2s[:], lss[:], AF.Exp)
            z2b = smpool.tile([128, 1], F32, tag="z2b")
            nc.vector.tensor_tensor(z2b[:], z2[:], p2s[:], op=OP.add)
            zc = smpool.tile([128, 1], F32, tag="zc")
            nc.vector.tensor_scalar_max(zc[:], z2b[:], 1e-30)
            rz2 = smpool.tile([128, 1], F32, tag="rz2")
            nc.vector.reciprocal(rz2[:], zc[:])
            # weighted messages in place, then reduce over slots
            nc.vector.tensor_tensor(
                gt[:, :, 0:NCLS], gt[:, :, 0:NCLS],
                aE[:, :, None].to_broadcast([128, K, NCLS]), op=OP.mult)
            nc.vector.tensor_tensor(
                gt[:, :, 64: 64 + NCLS], gt[:, :, 64: 64 + NCLS],
                aO[:, :, None].to_broadcast([128, K, NCLS]), op=OP.mult)
            u2a = smpool.tile([128, NCLS], F32, tag="u2a")
            nc.vector.tensor_reduce(
                u2a[:], gt[:, :, 0:NCLS].rearrange("p k f -> p f k"),
                axis=AX.X, op=OP.add)
            u2b = smpool.tile([128, NCLS], F32, tag="u2b")
            nc.vector.tensor_reduce(
                u2b[:], gt[:, :, 64: 64 + NCLS].rearrange("p k f -> p f k"),
                axis=AX.X, op=OP.add)
            u2 = smpool.tile([128, NCLS], F32, tag="u2")
            nc.vector.tensor_tensor(u2[:], u2a[:], u2b[:], op=OP.add)
            msel = smpool.tile([128, NCLS], F32, tag="msel")
            nc.vector.tensor_tensor(
                msel[:], tb2self[:, c, 0:NCLS],
                p2s[:].to_broadcast([128, NCLS]), op=OP.mult)
            u2c = smpool.tile([128, NCLS], F32, tag="u2c")
            nc.vector.tensor_tensor(u2c[:], u2[:], msel[:], op=OP.add)
            o2 = opool.tile([128, NCLS], F32, tag="o2")
            nc.vector.tensor_tensor(
                o2[:], u2c[:], rz2[:].to_broadcast([128, NCLS]), op=OP.mult)
            nc.sync.dma_start(t["out2"][128 * c: 128 * (c + 1), :], o2[:])


# ----------------------------------------------------------------------------
# PJRT execution (with on-device iteration chaining for timing)
# ----------------------------------------------------------------------------

def _pjrt_exec(nc, in_maps, n_cores, iters=1, reps=3):
    import jax
    import numpy as _np
    from jax.sharding import Mesh, PartitionSpec
    from jax.experimental.shard_map import shard_map
    from concourse import bass2jax as b2j
    from concourse import mybir as _mb

    b2j.install_neuronx_cc_hook()
    partition_name = (nc.partition_id_tensor.name
                      if nc.partition_id_tensor else None)
    in_names, out_names, out_avals, zero_outs = [], [], [], []
    for alloc in nc.m.functions[0].allocations:
        if not isinstance(alloc, _mb.MemoryLocationSet):
            continue
        name = alloc.memorylocations[0].name
        if alloc.kind == "ExternalInput":
            if name != partition_name:
                in_names.append(name)
        elif alloc.kind == "ExternalOutput":
            shape = tuple(alloc.tensor_shape)
            dtype = _mb.dt.np(alloc.dtype)
            out_names.append(name)
            out_avals.append(jax.core.ShapedArray(shape, dtype))
            zero_outs.append(_np.zeros(shape, dtype))
    n_params = len(in_names)
    all_in_names = in_names + out_names
    if partition_name is not None:
        all_in_names = all_in_names + [partition_name]

    def _body(*args):
        ins = list(args[:n_params])
        zo = list(args[n_params:])
        for _ in range(iters):
            operands = ins + zo
            if partition_name is not None:
                operands.append(b2j.partition_id_tensor())
            outs = _bass_exec_bind(b2j, operands, out_avals, all_in_names,
                                   out_names, nc)
            zo = list(outs)
        return tuple(zo)

    devices = jax.devices()[:n_cores]
    mesh = Mesh(_np.asarray(devices), ("core",))
    in_specs = (PartitionSpec("core"),) * (n_params + len(out_names))
    out_specs = (PartitionSpec("core"),) * len(out_names)
    sharded = jax.jit(shard_map(_body, mesh=mesh, in_specs=in_specs,
                                out_specs=out_specs, check_rep=False),
                      keep_unused=True)
    concat_in = [
        _np.concatenate([_np.asarray(in_maps[c][nm]) for c in range(n_cores)],
                        axis=0)
        for nm in in_names
    ]
    concat_zeros = [_np.zeros((n_cores * z.shape[0], *z.shape[1:]), z.dtype)
                    for z in zero_outs]
    import time as _time
    from jax.sharding import NamedSharding
    sh = NamedSharding(mesh, PartitionSpec("core"))
    dev_in = [jax.device_put(a, sh) for a in concat_in]
    dev_zeros = [jax.device_put(a, sh) for a in concat_zeros]
    jax.block_until_ready(dev_in + dev_zeros)
    out_arrs = sharded(*dev_in, *dev_zeros)
    jax.block_until_ready(out_arrs)
    times = []
    for _ in range(reps):
        t0 = _time.perf_counter()
        out_arrs = sharded(*dev_in, *dev_zeros)
        jax.block_until_ready(out_arrs)
        times.append(_time.perf_counter() - t0)
    dt = min(times)
    results = [
        {nm: _np.asarray(out_arrs[i]).reshape(n_cores, *out_avals[i].shape)[c]
         for i, nm in enumerate(out_names)}
        for c in range(n_cores)
    ]
    return results, dt


def _bass_exec_bind(b2j, operands, out_avals, in_names, out_names, nc):
    return b2j._bass_exec_p.bind(
        *operands,
        out_avals=tuple(out_avals),
        in_names=tuple(in_names),
        out_names=tuple(out_names),
        lowering_input_output_aliases=(),
        sim_require_finite=True,
        sim_require_nnan=True,
        nc=nc,
    )


# ----------------------------------------------------------------------------
# Entry point
# ----------------------------------------------------------------------------

_CACHE = {}


def _run(inputs, trace=False):
    x = np.asarray(inputs["x"], np.float32)
    edge_index = np.asarray(inputs["edge_index"], np.int32)
    W1 = np.asarray(inputs["W1"], np.float32)
    a1s = np.asarray(inputs["att1_src"], np.float32)
    a1d = np.asarray(inputs["att1_dst"], np.float32)
    W2 = np.asarray(inputs["W2"], np.float32)
    a2s = np.asarray(inputs["att2_src"], np.float32)
    a2d = np.asarray(inputs["att2_dst"], np.float32)
    b1 = np.asarray(inputs["b1"], np.float32)
    b2 = np.asarray(inputs["b2"], np.float32)
    assert not b1.any() and not b2.any(), "nonzero bias unsupported"

    key = hashlib.sha1(
        b"v3" + edge_index.tobytes() + np.int64(x.shape).tobytes()
    ).hexdigest()
    cfg, shared, per_core = _host_prep(x, edge_index, W1, a1s, a1d, W2, a2s, a2d)
    if key not in _CACHE:
        _CACHE[key] = _build_program(cfg)
    nc = _CACHE[key]

    in_maps = []
    for k in range(cfg["n_cores"]):
        m = dict(shared)
        m.update(per_core[k])
        in_maps.append(m)
    res = run_bass_kernel_spmd(nc, in_maps, list(range(cfg["n_cores"])),
                               trace=trace)
    out = gather_out([res.results[k]["out2"] for k in range(cfg["n_cores"])],
                     cfg)
    return out.astype(np.float32), res


def gather_out(outs, cfg):
    allrows = np.concatenate(outs, axis=0)          # [R, NCLS] permuted
    return allrows[cfg["pos"][: cfg["N"]]]


def kernel(**inputs):
    out, _ = _run(inputs, trace=False)
    return out


# revision 24
# speedup vs baseline: 2.1525x; 2.1525x over previous
"""GAT (2-layer, PyG-default) Trainium2 Bass kernel, 8-core SPMD.

v3 — trace-driven rework of the dst-major design (baseline 2.0ms ->
~1.56ms).  Measured constraints that shaped it: every gathered row is
one SWDGE descriptor costing ~70-90ns of SDMA-engine time regardless
of size (the kernel is descriptor-count-bound, ~230k descs/core), a
dma_gather call with >1024 indices wedges the device, calls with >512
descriptors block the GpSimd engine until ring space frees, and any
2-input DVE op holds the SBUF port pair that SWDGE descriptor
generation needs (DVE TENSOR_TENSOR time stalls the gather pipe).

  - Node permutation is globally degree-balanced: nodes ranked by
    in-degree are dealt into (chunk, core, lane) so each chunk's 1024
    lanes (128 per core x 8 cores) have near-uniform degree, shrinking
    the per-chunk slot count K toward the mean degree.
  - L1 rows are stored feature-transposed (c,h): the per-edge softmax
    weight broadcast then has unit inner stride on every operand, so
    the big per-round message multiply runs in DVE 2x_1P mode
    (in-place on the gather tile; no per-slot DVE ops).
  - L1 chunks are processed in SLAB-slot rounds: small gather tiles
    give a deep (6-buffer) gather pipeline, gather calls stay at <=4
    slots (512 descriptors, fire-and-forget), and each DVE-lock piece
    is ~3us so the 4 SWDGE queue rings (~9us of buffered descriptors)
    ride through it.  PSUM accumulates across rounds.
  - L2 gathers PAIRS of compact 128B tb2 rows (idx = src>>1, 256B
    descriptors): the pair index range (25088 < 32768) fits one int16
    window, killing L1's 4-window slot inflation for L2.  Even/odd row
    selection is folded into the alpha masks (me/mo).  L2 self-loop
    contributions come from an SBUF-resident tb2self captured while L1
    writes tb2 rows, so they never touch the gather path.
  - Phase 0 splits the PSUM->bf16 casts 3:1 between Vector and Scalar;
    transposes and the W2 projection run in bf16.

Self-contained: only needs numpy + the concourse tree at /opt/trn_rl_repo.
"""

import hashlib
import sys

import numpy as np

for _p in ("/opt/trn_rl_repo",):
    if _p not in sys.path:
        sys.path.insert(0, _p)

import concourse.bacc as bacc
import concourse.bass as bass
import concourse.tile as tile
from concourse import mybir
from concourse.bass_utils import run_bass_kernel_spmd

F32 = mybir.dt.float32
BF16 = mybir.dt.bfloat16
I16 = mybir.dt.int16
AF = mybir.ActivationFunctionType
OP = mybir.AluOpType
AX = mybir.AxisListType

N_CORES = 8
SPAN = 32768
W = 4
GMAX = 4
SLAB = 12  # L1 slots per gather tile / DVE-multiply piece (multiple of GMAX)
_QCTR = [0]  # global SWDGE queue round-robin


# ----------------------------------------------------------------------------
# Host-side edge planning
# ----------------------------------------------------------------------------

def _wrap_idx(si, n_cores, K):
    """[n_cores,128,K] int -> [n_cores,16,8K] in the dma_gather idx layout
    (idx of token T, partition p lands at [p%16, 8*T + p//16])."""
    tmp = si.reshape(n_cores, 8, 16, K)
    return np.ascontiguousarray(tmp.transpose(0, 2, 3, 1)).reshape(
        n_cores, 16, 8 * K)


def _edge_plan(edge_index, N, n_cores, nch, npcp):
    """Degree-balanced dst-major plans for both layers.

    L1: self-loops included, W=4 overlapping 32768-row windows.
    L2: self-loops excluded, rows gathered in PAIRS (idx = srow>>1) from
        a single window, with even/odd masks me/mo.
    """
    R = n_cores * npcp
    bases = [round(q * (R - SPAN) / (W - 1)) for q in range(W)]

    src1 = np.concatenate([np.asarray(edge_index[0], np.int64), np.arange(N)])
    dst1 = np.concatenate([np.asarray(edge_index[1], np.int64), np.arange(N)])
    deg = np.bincount(dst1, minlength=R)
    # Deal degree-ranked nodes into (chunk, core, lane): chunk lanes get
    # near-uniform degree across all cores.
    order = np.argsort(deg, kind="stable")
    blk = 128 * n_cores
    ii = np.arange(R)
    chunk = ii // blk
    core = (ii % blk) // 128
    lane = ii % 128
    pos = np.empty(R, np.int64)
    pos[order] = core * npcp + chunk * 128 + lane

    def build_emat(src, dst):
        srow = pos[src]
        dpos = pos[dst]
        key = dpos // npcp * (nch * 128) + dpos % npcp
        order_e = np.lexsort((srow, key))
        ks, ss = key[order_e], srow[order_e]
        counts = np.bincount(ks, minlength=n_cores * nch * 128)
        maxd = max(int(counts.max()), 1)
        starts = np.zeros(len(counts) + 1, np.int64)
        np.cumsum(counts, out=starts[1:])
        col = np.arange(len(ss)) - starts[ks]
        E = np.full((n_cores * nch * 128, maxd), 2**31, np.int64)
        E[ks, col] = ss
        return E, counts

    Emat1, counts1 = build_emat(src1, dst1)
    Emat2, counts2 = build_emat(np.asarray(edge_index[0], np.int64),
                                np.asarray(edge_index[1], np.int64))

    def plan_chunk(E, degv):
        valid = E < 2**31
        A = np.zeros(W, np.int64)
        B = np.zeros(W, np.int64)
        dmax = int(degv.max())
        for q in range(W - 1):
            A[q] = int(((E < bases[q + 1]) & valid).sum(axis=1).max())
            B[q] = int(((E >= bases[q] + SPAN) & valid).sum(axis=1).max())
        A[W - 1] = dmax
        K = int(max(dmax, (A + B).max(), 1))
        L = E.shape[0]
        maxd = E.shape[1]
        while True:
            P = np.maximum.accumulate(np.minimum(np.maximum(A, 0), K - B))
            P[W - 1] = K
            n = np.diff(np.concatenate([[0], P]))
            qcls = np.repeat(np.arange(W), n)
            ptr = np.zeros(L, np.int64)
            slotidx = np.zeros((L, K), np.int32)
            slotmask = np.zeros((L, K), bool)
            ok = True
            for t in range(K):
                b = bases[qcls[t]]
                cur = E[np.arange(L), np.minimum(ptr, maxd - 1)]
                vv = ptr < degv
                if np.any(vv & (cur < b)):
                    ok = False
                    break
                fit = vv & (cur >= b) & (cur < b + SPAN)
                slotidx[:, t] = np.where(fit, cur - b, 0)
                slotmask[:, t] = fit
                ptr += fit
            if ok and np.all(ptr == degv):
                return K, qcls, slotidx, slotmask
            K += 1
            assert K < dmax + 24, "edge window planning failed to converge"

    Ks, toff, calls = [], [], []
    blocks_idx, blocks_mask = [], []
    K2s, toff2 = [], []
    blocks_idx2, blocks_me, blocks_mo = [], [], []
    off = 0
    off2 = 0
    for c in range(nch):
        lanes = ((np.arange(n_cores)[:, None] * nch + c) * 128
                 + np.arange(128)[None, :]).ravel()
        # ---- L1 (windowed, self-loops in-plan) ----
        K, qcls, si, sm = plan_chunk(Emat1[lanes], counts1[lanes])
        Ks.append(K)
        toff.append(off)
        cc = []
        t0 = 0
        while t0 < K:
            q = qcls[t0]
            t1 = t0
            nxt_slab = (t0 // SLAB + 1) * SLAB
            while (t1 < K and qcls[t1] == q and t1 - t0 < GMAX
                   and t1 < nxt_slab):
                t1 += 1
            cc.append((t0, t1, int(q)))
            t0 = t1
        calls.append(cc)
        blocks_idx.append(_wrap_idx(si.reshape(n_cores, 128, K), n_cores, K))
        blocks_mask.append(sm.reshape(n_cores, 128, K))
        off += K
        # ---- L2 (paired rows, single window, no self-loops) ----
        E2 = Emat2[lanes]
        degv2 = counts2[lanes]
        K2 = max(int(degv2.max()), 1)
        sub = E2[:, :K2]
        valid = np.arange(K2)[None, :] < degv2[:, None]
        idx2 = np.where(valid, sub >> 1, 0).astype(np.int32)
        par = np.where(valid, sub & 1, 0)
        me = (valid & (par == 0)).astype(np.float32)
        mo = (valid & (par == 1)).astype(np.float32)
        K2s.append(K2)
        toff2.append(off2)
        blocks_idx2.append(_wrap_idx(idx2.reshape(n_cores, 128, K2),
                                     n_cores, K2))
        blocks_me.append(me.reshape(n_cores, 128, K2))
        blocks_mo.append(mo.reshape(n_cores, 128, K2))
        off2 += K2
    TOT, TOT2 = off, off2
    idx16 = np.tile(np.concatenate(blocks_idx, axis=2).astype(np.int16),
                    (1, 8, 1))
    mask = np.concatenate(blocks_mask, axis=2).astype(np.float32)
    idx16_2 = np.tile(np.concatenate(blocks_idx2, axis=2).astype(np.int16),
                      (1, 8, 1))
    me_all = np.concatenate(blocks_me, axis=2)
    mo_all = np.concatenate(blocks_mo, axis=2)
    return (pos, bases, Ks, toff, TOT, calls, idx16, mask,
            K2s, toff2, TOT2, idx16_2, me_all, mo_all)


def _host_prep(x, edge_index, W1, att1_src, att1_dst, W2, att2_src, att2_dst):
    N, F = x.shape
    H, C = att1_src.shape
    HC = H * C
    NCLS = W2.shape[1]
    n_cores = N_CORES
    nch = -(-N // (n_cores * 128))
    npcp = nch * 128
    R = n_cores * npcp

    (pos, bases, Ks, toff, TOT, calls, idx16, mask,
     K2s, toff2, TOT2, idx16_2, me_all, mo_all) = _edge_plan(
        edge_index, N, n_cores, nch, npcp)

    # (c,h) feature permutation: new col c*H+h <- old col h*C+c
    jj = np.arange(HC)
    perm = (jj % H) * C + jj // H
    W1p = W1[:, perm]

    # Folded attention-logit weight columns (independent of column order)
    Wa_s = np.einsum("fhc,hc->fh", W1.reshape(F, H, C), att1_src).astype(np.float32)
    Wa_d = np.einsum("fhc,hc->fh", W1.reshape(F, H, C), att1_dst).astype(np.float32)
    W1e = np.ascontiguousarray(
        np.concatenate([W1p, Wa_s, Wa_d], axis=1), dtype=np.float32)  # [F, 528]

    w2s = (W2 @ att2_src[0]).astype(np.float32)
    w2d = (W2 @ att2_dst[0]).astype(np.float32)
    W2e_flat = np.zeros((HC, 64), np.float32)
    W2e_flat[:, :NCLS] = W2[perm]
    W2e_flat[:, NCLS] = w2s[perm]
    W2e_flat[:, NCLS + 1] = w2d[perm]
    nslab = HC // 128
    W2e = np.ascontiguousarray(
        W2e_flat.reshape(nslab, 128, 64).transpose(1, 0, 2))  # [128, 4, 64]

    import ml_dtypes
    bf = ml_dtypes.bfloat16
    xtab = np.zeros((R, F), np.float32)
    xtab[pos[np.arange(N)]] = x
    xTp = np.ascontiguousarray(xtab.T).astype(bf)   # [F, R] permuted cols
    W1e = W1e.astype(bf)
    ident = np.eye(128, dtype=np.float32).astype(bf)

    cfg = dict(
        N=N, F=F, H=H, C=C, HC=HC, NCLS=NCLS, n_cores=n_cores,
        nch=nch, npcp=npcp, R=R, nslab=nslab,
        Ks=Ks, toff=toff, TOT=TOT, calls=calls, bases=bases, pos=pos,
        K2s=K2s, toff2=toff2, TOT2=TOT2,
        swdge_queues=4, p0_bufs=4,
        gt_bufs=6, g2_bufs=4,
    )
    shared = dict(xTp=xTp, W1e=W1e, W2e=W2e.astype(bf), ident=ident)
    per_core = [
        dict(g1idx=idx16[k], mask=mask[k].astype(bf),
             g2idx=idx16_2[k], me=me_all[k], mo=mo_all[k])
        for k in range(n_cores)
    ]
    return cfg, shared, per_core


# ----------------------------------------------------------------------------
# Device program
# ----------------------------------------------------------------------------

def _build_program(cfg):
    F, HC, NCLS = cfg["F"], cfg["HC"], cfg["NCLS"]
    n_cores, npcp, R = cfg["n_cores"], cfg["npcp"], cfg["R"]
    nslab, TOT, TOT2 = cfg["nslab"], cfg["TOT"], cfg["TOT2"]

    nc = bacc.Bacc("TRN2", target_bir_lowering=False, debug=False,
                   num_devices=n_cores,
                   num_swdge_queues=cfg.get("swdge_queues", 1))

    xTp = nc.dram_tensor("xTp", [F, R], BF16, kind="ExternalInput").ap()
    W1e = nc.dram_tensor("W1e", [F, HC + 16], BF16, kind="ExternalInput").ap()
    W2e = nc.dram_tensor("W2e", [128, nslab, 64], BF16, kind="ExternalInput").ap()
    ident_d = nc.dram_tensor("ident", [128, 128], BF16, kind="ExternalInput").ap()
    g1idx = nc.dram_tensor("g1idx", [128, 8 * TOT], I16,
                           kind="ExternalInput").ap()
    mask_d = nc.dram_tensor("mask", [128, TOT], BF16, kind="ExternalInput").ap()
    g2idx = nc.dram_tensor("g2idx", [128, 8 * TOT2], I16,
                           kind="ExternalInput").ap()
    me_d = nc.dram_tensor("me", [128, TOT2], F32, kind="ExternalInput").ap()
    mo_d = nc.dram_tensor("mo", [128, TOT2], F32, kind="ExternalInput").ap()

    T1 = nc.dram_tensor("T1", [R, 640], BF16).ap()
    tb2_own = nc.dram_tensor("tb2_own", [npcp, 64], BF16).ap()
    tb2_full = nc.dram_tensor("tb2_full", [R, 64], BF16,
                              addr_space="Shared").ap()
    out2 = nc.dram_tensor("out2", [npcp, NCLS], F32, kind="ExternalOutput").ap()

    tensors = dict(xTp=xTp, W1e=W1e, W2e=W2e, ident=ident_d, g1idx=g1idx,
                   mask=mask_d, g2idx=g2idx, me=me_d, mo=mo_d,
                   T1=T1, tb2_own=tb2_own, tb2_full=tb2_full, out2=out2)
    repeat = cfg.get("repeat", 1)
    with tile.TileContext(nc) as tc:
        for _ in range(repeat):
            _emit(tc, cfg, tensors)
    nc.compile()
    return nc


def _emit(tc, cfg, t):
    nc = tc.nc
    H, HC, NCLS = cfg["H"], cfg["HC"], cfg["NCLS"]
    n_cores, nch, npcp, R = cfg["n_cores"], cfg["nch"], cfg["npcp"], cfg["R"]
    nslab = cfg["nslab"]
    NTB = R // 128

    with tc.tile_pool(name="consts", bufs=1) as cpool:
        W1e_sb = cpool.tile([128, HC + 16], BF16)
        nc.sync.dma_start(W1e_sb[:], t["W1e"][:, :])
        W2e_sb = cpool.tile([128, nslab, 64], BF16)
        nc.sync.dma_start(W2e_sb[:], t["W2e"][:, :, :])
        ident_bf = cpool.tile([128, 128], BF16)
        nc.sync.dma_start(ident_bf[:], t["ident"][:, :])
        ald1_all = cpool.tile([128, NTB, H], F32)
        ald1_sb = cpool.tile([128, nch, H], F32)
        tb2self = cpool.tile([128, nch, 64], BF16)

        # ---------------- Phase 0: permuted node table T1 ----------------
        with (
            nc.named_scope("p0"),
            tc.tile_pool(name="p0", bufs=cfg.get("p0_bufs", 4)) as pool,
            tc.tile_pool(name="p0ps", bufs=cfg.get("p0_bufs", 4),
                         space="PSUM") as pps,
        ):
            nblk = R // 512
            for i in range(nblk):
                xt = pool.tile([128, 512], BF16, tag="xt")
                nc.sync.dma_start(xt[:], t["xTp"][:, 512 * i: 512 * i + 512])
                rowB = pool.tile([128, 4, HC + 16], BF16, tag="rowB")
                for j in range(4):
                    # two matmuls: a 512-col PSUM write must stay in one bank
                    ps = pps.tile([128, HC + 16], F32, tag="ps")
                    nc.tensor.matmul(ps[:, 0:HC],
                                     lhsT=xt[:, 128 * j: 128 * j + 128],
                                     rhs=W1e_sb[:, 0:HC], start=True, stop=True)
                    nc.tensor.matmul(ps[:, HC: HC + 16],
                                     lhsT=xt[:, 128 * j: 128 * j + 128],
                                     rhs=W1e_sb[:, HC: HC + 16],
                                     start=True, stop=True)
                    if j < 3:
                        nc.vector.tensor_copy(rowB[:, j, 0:HC], ps[:, 0:HC])
                    else:
                        nc.scalar.copy(rowB[:, j, 0:HC], ps[:, 0:HC])
                    nc.vector.tensor_copy(
                        rowB[:, j, HC: HC + 16].bitcast(F32),
                        ps[:, HC: HC + H])
                    nc.scalar.copy(ald1_all[:, 4 * i + j, :],
                                   ps[:, HC + H: HC + 2 * H])
                # ACT HWDGE ring: don't serialize behind xt reads on SP ring
                nc.scalar.dma_start(
                    t["T1"][512 * i: 512 * i + 512, 0: HC + 16].rearrange(
                        "(j p) c -> p j c", p=128),
                    rowB[:],
                )

        pid = nc.partition_id()
        nc.sync.dma_start(ald1_sb[:], ald1_all[:, bass.ds(pid * nch, nch), :])

        if cfg.get("phases", "full") == "p0":
            return
        # ---------------- L1 edge phase ----------------
        with nc.named_scope("l1"):
            _l1_phase(tc, cfg, t, W2e_sb, ident_bf, ald1_sb, tb2self)

        if cfg.get("phases", "full") == "p0+l1":
            return
        # ---------------- allgather ----------------
        with nc.named_scope("ag"):
            if cfg.get("no_collective"):
                nc.sync.dma_start(t["tb2_full"][0:npcp, :], t["tb2_own"][:, :])
            else:
                nc.gpsimd.collective_compute(
                    "AllGather",
                    OP.bypass,
                    replica_groups=[list(range(n_cores))],
                    ins=[t["tb2_own"][:, :]],
                    outs=[t["tb2_full"][:, :]],
                )

        if cfg.get("phases", "full") == "p0+l1+ag":
            return
        # ---------------- L2 edge phase ----------------
        with nc.named_scope("l2"):
            _l2_phase(tc, cfg, t, tb2self)


def _l1_phase(tc, cfg, t, W2e_sb, ident_bf, ald1_sb, tb2self):
    nc = tc.nc
    nch, H, C, HC, NCLS = cfg["nch"], cfg["H"], cfg["C"], cfg["HC"], cfg["NCLS"]
    nslab = cfg["nslab"]
    Ks, toff, calls, bases = cfg["Ks"], cfg["toff"], cfg["calls"], cfg["bases"]
    TOT = cfg["TOT"]
    nq = cfg.get("swdge_queues", 1)

    with (
        tc.tile_pool(name="gt1", bufs=cfg.get("gt_bufs", 2)) as gpool,
        tc.tile_pool(name="meta1", bufs=1) as mpool,
        tc.tile_pool(name="small1", bufs=3) as smpool,
        tc.tile_pool(name="out1", bufs=2) as opool,
        tc.tile_pool(name="ps_u1", bufs=2, space="PSUM") as pp_u,
        tc.tile_pool(name="ps_tr1", bufs=2, space="PSUM") as pp_tr,
        tc.tile_pool(name="ps_o1", bufs=2, space="PSUM") as pp_o,
        tc.tile_pool(name="ps_a1", bufs=2, space="PSUM") as pp_a,
    ):
        idx_all = mpool.tile([128, 8 * TOT], I16, tag="idxall")
        nc.sync.dma_start(idx_all[:], t["g1idx"][:, :])
        msk_all = mpool.tile([128, TOT], BF16, tag="mskall")
        nc.sync.dma_start(msk_all[:], t["mask"][:, :])
        for c in range(nch):
            K = Ks[c]
            off = toff[c]
            stop = cfg.get("l1_stop")
            p_bf = smpool.tile([128, K, H], BF16, tag="pbf")
            ps_u = pp_u.tile([128, HC], F32, tag="u")
            # slab rounds: small gather tiles (deep pipelining) and short
            # DVE-lock pieces so SWDGE descriptor generation stays fed
            for r0 in range(0, K, SLAB):
                r1 = min(r0 + SLAB, K)
                kr = r1 - r0
                gt = gpool.tile([128, SLAB, 640], BF16, tag="gt")
                for (b0, b1, q) in calls[c]:
                    if b0 < r0 or b0 >= r1:
                        continue
                    nk = b1 - b0
                    nc.gpsimd.dma_gather(
                        gt[:, b0 - r0: b1 - r0, :],
                        t["T1"][bases[q]: bases[q] + SPAN, :],
                        idx_all[:, 8 * (off + b0): 8 * (off + b1)],
                        nk * 128, nk * 128, 640,
                        queue_num=_QCTR[0] % nq,
                    )
                    _QCTR[0] += 1
                if stop == "gather":
                    continue
                # p = exp(leakyrelu(al_src[src] + al_dst[dst])) * mask
                s_t = smpool.tile([128, SLAB, H], F32, tag="s")
                nc.vector.tensor_tensor(
                    s_t[:, 0:kr, :],
                    gt[:, 0:kr, HC: HC + 16].bitcast(F32),
                    ald1_sb[:, c, None, :].to_broadcast([128, kr, H]),
                    op=OP.add,
                )
                l_t = smpool.tile([128, SLAB, H], F32, tag="l")
                nc.vector.scalar_tensor_tensor(
                    l_t[:, 0:kr, :], s_t[:, 0:kr, :], 0.2, s_t[:, 0:kr, :],
                    op0=OP.mult, op1=OP.max
                )
                p_t = smpool.tile([128, SLAB, H], F32, tag="p")
                nc.scalar.activation(p_t[:, 0:kr, :], l_t[:, 0:kr, :], AF.Exp)
                nc.vector.tensor_tensor(
                    p_bf[:, r0:r1, :], p_t[:, 0:kr, :],
                    msk_all[:, off + r0: off + r1, None].to_broadcast(
                        [128, kr, H]),
                    op=OP.mult,
                )
                if stop in ("logits", "mult", "agg"):
                    continue
                # in-place alpha multiply, (c,h) layout -> unit inner stride.
                # alpha staged into PSUM via ACT: the TT then holds only one
                # SBUF read port, so SWDGE descriptor-gen is not locked out.
                if cfg.get("alpha_psum", False):
                    pa = pp_a.tile([128, SLAB, H], BF16, tag="pa")
                    nc.scalar.copy(pa[:, 0:kr, :], p_bf[:, r0:r1, :])
                    alpha_src = pa[:, 0:kr, None, :]
                else:
                    alpha_src = p_bf[:, r0:r1, None, :]
                nc.vector.tensor_tensor(
                    gt[:, 0:kr, 0:HC].rearrange("p k (c h) -> p k c h", h=H),
                    gt[:, 0:kr, 0:HC].rearrange("p k (c h) -> p k c h", h=H),
                    alpha_src.to_broadcast([128, kr, C, H]),
                    op=OP.mult,
                )
                for k in range(kr):
                    nc.tensor.matmul(
                        ps_u[:], lhsT=ident_bf[:], rhs=gt[:, k, 0:HC],
                        start=(r0 + k == 0), stop=(r0 + k == K - 1),
                    )
            if stop in ("gather", "logits", "mult", "agg"):
                continue
            zr = smpool.tile([128, H], F32, tag="zr")
            nc.vector.tensor_reduce(
                zr[:], p_bf[:].rearrange("p k h -> p h k"),
                axis=AX.X, op=OP.add,
            )
            zb = smpool.tile([128, H], F32, tag="zb")
            nc.vector.tensor_scalar_max(zb[:], zr[:], 1e-30)
            rz = smpool.tile([128, H], F32, tag="rz")
            nc.vector.reciprocal(rz[:], zb[:])
            h2 = opool.tile([128, HC], F32, tag="h2")
            nc.vector.tensor_tensor(
                h2[:].rearrange("p (c h) -> p c h", h=H),
                ps_u[:].rearrange("p (c h) -> p c h", h=H),
                rz[:, None, :].to_broadcast([128, C, H]),
                op=OP.mult,
            )
            h2r = opool.tile([128, HC], BF16, tag="h2r")
            nc.scalar.activation(h2r[:], h2[:], AF.Relu)
            if stop == "h2":
                continue
            ps_o = pp_o.tile([128, 64], F32, tag="o")
            for j in range(nslab):
                ps_tr = pp_tr.tile([128, 128], BF16, tag="tr")
                nc.tensor.transpose(
                    ps_tr[:], h2r[:, 128 * j: 128 * (j + 1)], ident_bf[:]
                )
                h2t = smpool.tile([128, 128], BF16, tag="h2t")
                nc.scalar.copy(h2t[:], ps_tr[:])
                nc.tensor.matmul(
                    ps_o[:], lhsT=h2t[:], rhs=W2e_sb[:, j, :],
                    start=(j == 0), stop=(j == nslab - 1),
                )
            # tb2 row: [40 cls bf16 | al2_src f32 pair | al2_dst f32 pair | pad]
            nc.vector.tensor_copy(tb2self[:, c, 0:NCLS], ps_o[:, 0:NCLS])
            nc.scalar.copy(tb2self[:, c, NCLS: NCLS + 4].bitcast(F32),
                           ps_o[:, NCLS: NCLS + 2])
            nc.sync.dma_start(t["tb2_own"][128 * c: 128 * (c + 1), :],
                              tb2self[:, c, :])


def _l2_phase(tc, cfg, t, tb2self):
    nc = tc.nc
    nch, NCLS = cfg["nch"], cfg["NCLS"]
    K2s, toff2, TOT2 = cfg["K2s"], cfg["toff2"], cfg["TOT2"]
    nq = cfg.get("swdge_queues", 1)

    with (
        tc.tile_pool(name="gt2", bufs=cfg.get("g2_bufs", 3)) as gpool,
        tc.tile_pool(name="meta2", bufs=1) as mpool,
        tc.tile_pool(name="small2", bufs=3) as smpool,
        tc.tile_pool(name="out2p", bufs=2) as opool,
    ):
        idx_all = mpool.tile([128, 8 * TOT2], I16, tag="idx2all")
        nc.sync.dma_start(idx_all[:], t["g2idx"][:, :])
        me_all = mpool.tile([128, TOT2], F32, tag="meall")
        nc.sync.dma_start(me_all[:], t["me"][:, :])
        mo_all = mpool.tile([128, TOT2], F32, tag="moall")
        nc.sync.dma_start(mo_all[:], t["mo"][:, :])
        tb2p = t["tb2_full"].rearrange("(a b) c -> a (b c)", b=2)
        for c in range(nch):
            K = K2s[c]
            off = toff2[c]
            me = me_all[:, off: off + K]
            mo = mo_all[:, off: off + K]
            gt = gpool.tile([128, K, 128], BF16, tag="gt2")
            for b0 in range(0, K, GMAX):
                b1 = min(b0 + GMAX, K)
                nk = b1 - b0
                nc.gpsimd.dma_gather(
                    gt[:, b0:b1, :], tb2p[:, :],
                    idx_all[:, 8 * (off + b0): 8 * (off + b1)],
                    nk * 128, nk * 128, 128,
                    queue_num=_QCTR[0] % nq,
                )
                _QCTR[0] += 1
            # logits: select even/odd al2_src, add own al2_dst
            ald = tb2self[:, c, NCLS + 2: NCLS + 4].bitcast(F32)  # [128,1]
            v1 = smpool.tile([128, K], F32, tag="v1")
            nc.vector.tensor_tensor(
                v1[:], gt[:, :, NCLS: NCLS + 2].bitcast(F32)[:, :, 0], me,
                op=OP.mult)
            v2 = smpool.tile([128, K], F32, tag="v2")
            nc.vector.tensor_tensor(
                v2[:], gt[:, :, 64 + NCLS: 64 + NCLS + 2].bitcast(F32)[:, :, 0],
                mo, op=OP.mult)
            s2 = smpool.tile([128, K], F32, tag="s2")
            nc.vector.tensor_tensor(s2[:], v1[:], v2[:], op=OP.add)
            s2b = smpool.tile([128, K], F32, tag="s2b")
            nc.vector.tensor_tensor(
                s2b[:], s2[:], ald.to_broadcast([128, K]), op=OP.add)
            l2t = smpool.tile([128, K], F32, tag="l2")
            nc.vector.scalar_tensor_tensor(
                l2t[:], s2b[:], 0.2, s2b[:], op0=OP.mult, op1=OP.max)
            p2 = smpool.tile([128, K], F32, tag="p2")
            nc.scalar.activation(p2[:], l2t[:], AF.Exp)
            aE = smpool.tile([128, K], BF16, tag="aE")
            nc.vector.tensor_tensor(aE[:], p2[:], me, op=OP.mult)
            aO = smpool.tile([128, K], BF16, tag="aO")
            nc.vector.tensor_tensor(aO[:], p2[:], mo, op=OP.mult)
            # z from the already-masked weights (1-src reduces, no port lock)
            zE = smpool.tile([128, 1], F32, tag="zE")
            nc.vector.tensor_reduce(zE[:], aE[:], axis=AX.X, op=OP.add)
            zO = smpool.tile([128, 1], F32, tag="zO")
            nc.vector.tensor_reduce(zO[:], aO[:], axis=AX.X, op=OP.add)
            z2 = smpool.tile([128, 1], F32, tag="z2")
            nc.vector.tensor_tensor(z2[:], zE[:], zO[:], op=OP.add)
            # self-loop: logit from own al2_src + own al2_dst
            ss = smpool.tile([128, 1], F32, tag="ss")
            nc.vector.tensor_tensor(
                ss[:], tb2self[:, c, NCLS: NCLS + 2].bitcast(F32), ald,
                op=OP.add)
            lss = smpool.tile([128, 1], F32, tag="lss")
            nc.vector.scalar_tensor_tensor(
                lss[:], ss[:], 0.2, ss[:], op0=OP.mult, op1=OP.max)
            p2s = smpool.tile([128, 1], F32, tag="p2s")
            nc.scalar.activation(p2s[:], lss[:], AF.Exp)
            z2b = smpool.tile([128, 1], F32, tag="z2b")
            nc.vector.tensor_tensor(z2b[:], z2[:], p2s[:], op=OP.add)
            zc = smpool.tile([128, 1], F32, tag="zc")
            nc.vector.tensor_scalar_max(zc[:], z2b[:], 1e-30)
            rz2 = smpool.tile([128, 1], F32, tag="rz2")
            nc.vector.reciprocal(rz2[:], zc[:])
            # weighted messages in place, then reduce over slots
            nc.vector.tensor_tensor(
                gt[:, :, 0:NCLS], gt[:, :, 0:NCLS],
                aE[:, :, None].to_broadcast([128, K, NCLS]), op=OP.mult)
            nc.vector.tensor_tensor(
                gt[:, :, 64: 64 + NCLS], gt[:, :, 64: 64 + NCLS],
                aO[:, :, None].to_broadcast([128, K, NCLS]), op=OP.mult)
            u2a = smpool.tile([128, NCLS], F32, tag="u2a")
            nc.vector.tensor_reduce(
                u2a[:], gt[:, :, 0:NCLS].rearrange("p k f -> p f k"),
                axis=AX.X, op=OP.add)
            u2b = smpool.tile([128, NCLS], F32, tag="u2b")
            nc.vector.tensor_reduce(
                u2b[:], gt[:, :, 64: 64 + NCLS].rearrange("p k f -> p f k"),
                axis=AX.X, op=OP.add)
            u2 = smpool.tile([128, NCLS], F32, tag="u2")
            nc.vector.tensor_tensor(u2[:], u2a[:], u2b[:], op=OP.add)
            msel = smpool.tile([128, NCLS], F32, tag="msel")
            nc.vector.tensor_tensor(
                msel[:], tb2self[:, c, 0:NCLS],
                p2s[:].to_broadcast([128, NCLS]), op=OP.mult)
            u2c = smpool.tile([128, NCLS], F32, tag="u2c")
            nc.vector.tensor_tensor(u2c[:], u2[:], msel[:], op=OP.add)
            o2 = opool.tile([128, NCLS], F32, tag="o2")
            nc.vector.tensor_tensor(
                o2[:], u2c[:], rz2[:].to_broadcast([128, NCLS]), op=OP.mult)
            nc.sync.dma_start(t["out2"][128 * c: 128 * (c + 1), :], o2[:])


# ----------------------------------------------------------------------------
# PJRT execution (with on-device iteration chaining for timing)
# ----------------------------------------------------------------------------

def _pjrt_exec(nc, in_maps, n_cores, iters=1, reps=3):
    import jax
    import numpy as _np
    from jax.sharding import Mesh, PartitionSpec
    from jax.experimental.shard_map import shard_map
    from concourse import bass2jax as b2j
    from concourse import mybir as _mb

    b2j.install_neuronx_cc_hook()
    partition_name = (nc.partition_id_tensor.name
                      if nc.partition_id_tensor else None)
    in_names, out_names, out_avals, zero_outs = [], [], [], []
    for alloc in nc.m.functions[0].allocations:
        if not isinstance(alloc, _mb.MemoryLocationSet):
            continue
        name = alloc.memorylocations[0].name
        if alloc.kind == "ExternalInput":
            if name != partition_name:
                in_names.append(name)
        elif alloc.kind == "ExternalOutput":
            shape = tuple(alloc.tensor_shape)
            dtype = _mb.dt.np(alloc.dtype)
            out_names.append(name)
            out_avals.append(jax.core.ShapedArray(shape, dtype))
            zero_outs.append(_np.zeros(shape, dtype))
    n_params = len(in_names)
    all_in_names = in_names + out_names
    if partition_name is not None:
        all_in_names = all_in_names + [partition_name]

    def _body(*args):
        ins = list(args[:n_params])
        zo = list(args[n_params:])
        for _ in range(iters):
            operands = ins + zo
            if partition_name is not None:
                operands.append(b2j.partition_id_tensor())
            outs = _bass_exec_bind(b2j, operands, out_avals, all_in_names,
                                   out_names, nc)
            zo = list(outs)
        return tuple(zo)

    devices = jax.devices()[:n_cores]
    mesh = Mesh(_np.asarray(devices), ("core",))
    in_specs = (PartitionSpec("core"),) * (n_params + len(out_names))
    out_specs = (PartitionSpec("core"),) * len(out_names)
    sharded = jax.jit(shard_map(_body, mesh=mesh, in_specs=in_specs,
                                out_specs=out_specs, check_rep=False),
                      keep_unused=True)
    concat_in = [
        _np.concatenate([_np.asarray(in_maps[c][nm]) for c in range(n_cores)],
                        axis=0)
        for nm in in_names
    ]
    concat_zeros = [_np.zeros((n_cores * z.shape[0], *z.shape[1:]), z.dtype)
                    for z in zero_outs]
    import time as _time
    from jax.sharding import NamedSharding
    sh = NamedSharding(mesh, PartitionSpec("core"))
    dev_in = [jax.device_put(a, sh) for a in concat_in]
    dev_zeros = [jax.device_put(a, sh) for a in concat_zeros]
    jax.block_until_ready(dev_in + dev_zeros)
    out_arrs = sharded(*dev_in, *dev_zeros)
    jax.block_until_ready(out_arrs)
    times = []
    for _ in range(reps):
        t0 = _time.perf_counter()
        out_arrs = sharded(*dev_in, *dev_zeros)
        jax.block_until_ready(out_arrs)
        times.append(_time.perf_counter() - t0)
    dt = min(times)
    results = [
        {nm: _np.asarray(out_arrs[i]).reshape(n_cores, *out_avals[i].shape)[c]
         for i, nm in enumerate(out_names)}
        for c in range(n_cores)
    ]
    return results, dt


def _bass_exec_bind(b2j, operands, out_avals, in_names, out_names, nc):
    return b2j._bass_exec_p.bind(
        *operands,
        out_avals=tuple(out_avals),
        in_names=tuple(in_names),
        out_names=tuple(out_names),
        lowering_input_output_aliases=(),
        sim_require_finite=True,
        sim_require_nnan=True,
        nc=nc,
    )


# ----------------------------------------------------------------------------
# Entry point
# ----------------------------------------------------------------------------

_CACHE = {}


def _run(inputs, trace=False):
    x = np.asarray(inputs["x"], np.float32)
    edge_index = np.asarray(inputs["edge_index"], np.int32)
    W1 = np.asarray(inputs["W1"], np.float32)
    a1s = np.asarray(inputs["att1_src"], np.float32)
    a1d = np.asarray(inputs["att1_dst"], np.float32)
    W2 = np.asarray(inputs["W2"], np.float32)
    a2s = np.asarray(inputs["att2_src"], np.float32)
    a2d = np.asarray(inputs["att2_dst"], np.float32)
    b1 = np.asarray(inputs["b1"], np.float32)
    b2 = np.asarray(inputs["b2"], np.float32)
    assert not b1.any() and not b2.any(), "nonzero bias unsupported"

    key = hashlib.sha1(
        b"v3" + edge_index.tobytes() + np.int64(x.shape).tobytes()
    ).hexdigest()
    cfg, shared, per_core = _host_prep(x, edge_index, W1, a1s, a1d, W2, a2s, a2d)
    if key not in _CACHE:
        _CACHE[key] = _build_program(cfg)
    nc = _CACHE[key]

    in_maps = []
    for k in range(cfg["n_cores"]):
        m = dict(shared)
        m.update(per_core[k])
        in_maps.append(m)
    res = run_bass_kernel_spmd(nc, in_maps, list(range(cfg["n_cores"])),
                               trace=trace)
    out = gather_out([res.results[k]["out2"] for k in range(cfg["n_cores"])],
                     cfg)
    return out.astype(np.float32), res


def gather_out(outs, cfg):
    allrows = np.concatenate(outs, axis=0)          # [R, NCLS] permuted
    return allrows[cfg["pos"][: cfg["N"]]]


def kernel(**inputs):
    out, _ = _run(inputs, trace=False)
    return out


# revision 27
# speedup vs baseline: 2.2810x; 1.0597x over previous
"""GAT (2-layer, PyG-default) Trainium2 Bass kernel, 8-core SPMD.

v3 — trace-driven rework of the dst-major design (baseline 2.0ms ->
~1.56ms).  Measured constraints that shaped it: every gathered row is
one SWDGE descriptor costing ~70-90ns of SDMA-engine time regardless
of size (the kernel is descriptor-count-bound, ~230k descs/core), a
dma_gather call with >1024 indices wedges the device, calls with >512
descriptors block the GpSimd engine until ring space frees, and any
2-input DVE op holds the SBUF port pair that SWDGE descriptor
generation needs (DVE TENSOR_TENSOR time stalls the gather pipe).

  - Node permutation is globally degree-balanced: nodes ranked by
    in-degree are dealt into (chunk, core, lane) so each chunk's 1024
    lanes (128 per core x 8 cores) have near-uniform degree, shrinking
    the per-chunk slot count K toward the mean degree.
  - L1 rows are stored feature-transposed (c,h): the per-edge softmax
    weight broadcast then has unit inner stride on every operand, so
    the big per-round message multiply runs in DVE 2x_1P mode
    (in-place on the gather tile; no per-slot DVE ops).
  - L1 chunks are processed in SLAB-slot rounds: small gather tiles
    give a deep (6-buffer) gather pipeline, gather calls stay at <=4
    slots (512 descriptors, fire-and-forget), and each DVE-lock piece
    is ~3us so the 4 SWDGE queue rings (~9us of buffered descriptors)
    ride through it.  PSUM accumulates across rounds.
  - L2 gathers PAIRS of compact 128B tb2 rows (idx = src>>1, 256B
    descriptors): the pair index range (25088 < 32768) fits one int16
    window, killing L1's 4-window slot inflation for L2.  Even/odd row
    selection is folded into the alpha masks (me/mo).  L2 self-loop
    contributions come from an SBUF-resident tb2self captured while L1
    writes tb2 rows, so they never touch the gather path.
  - Phase 0 splits the PSUM->bf16 casts 3:1 between Vector and Scalar;
    transposes and the W2 projection run in bf16.

Self-contained: only needs numpy + the concourse tree at /opt/trn_rl_repo.
"""

import hashlib
import sys

import numpy as np

for _p in ("/opt/trn_rl_repo",):
    if _p not in sys.path:
        sys.path.insert(0, _p)

import concourse.bacc as bacc
import concourse.bass as bass
import concourse.tile as tile
from concourse import mybir
from concourse.bass_utils import run_bass_kernel_spmd

F32 = mybir.dt.float32
BF16 = mybir.dt.bfloat16
I16 = mybir.dt.int16
AF = mybir.ActivationFunctionType
OP = mybir.AluOpType
AX = mybir.AxisListType

N_CORES = 8
SPAN = 32768
W = 4
GMAX = 4
SLAB = 12  # L1 slots per gather tile / DVE-multiply piece (multiple of GMAX)
_QCTR = [0]  # global SWDGE queue round-robin


# ----------------------------------------------------------------------------
# Host-side edge planning
# ----------------------------------------------------------------------------

def _wrap_idx(si, n_cores, K):
    """[n_cores,128,K] int -> [n_cores,16,8K] in the dma_gather idx layout
    (idx of token T, partition p lands at [p%16, 8*T + p//16])."""
    tmp = si.reshape(n_cores, 8, 16, K)
    return np.ascontiguousarray(tmp.transpose(0, 2, 3, 1)).reshape(
        n_cores, 16, 8 * K)


def _edge_plan(edge_index, N, n_cores, nch, npcp):
    """Degree-balanced dst-major plans for both layers.

    L1: self-loops included, W=4 overlapping 32768-row windows.
    L2: self-loops excluded, rows gathered in PAIRS (idx = srow>>1) from
        a single window, with even/odd masks me/mo.
    """
    R = n_cores * npcp
    bases = [round(q * (R - SPAN) / (W - 1)) for q in range(W)]

    src1 = np.concatenate([np.asarray(edge_index[0], np.int64), np.arange(N)])
    dst1 = np.concatenate([np.asarray(edge_index[1], np.int64), np.arange(N)])
    deg = np.bincount(dst1, minlength=R)
    # Deal degree-ranked nodes into (chunk, core, lane): chunk lanes get
    # near-uniform degree across all cores.
    order = np.argsort(deg, kind="stable")
    blk = 128 * n_cores
    ii = np.arange(R)
    chunk = ii // blk
    core = (ii % blk) // 128
    lane = ii % 128
    pos = np.empty(R, np.int64)
    pos[order] = core * npcp + chunk * 128 + lane

    def build_emat(src, dst):
        srow = pos[src]
        dpos = pos[dst]
        key = dpos // npcp * (nch * 128) + dpos % npcp
        order_e = np.lexsort((srow, key))
        ks, ss = key[order_e], srow[order_e]
        counts = np.bincount(ks, minlength=n_cores * nch * 128)
        maxd = max(int(counts.max()), 1)
        starts = np.zeros(len(counts) + 1, np.int64)
        np.cumsum(counts, out=starts[1:])
        col = np.arange(len(ss)) - starts[ks]
        E = np.full((n_cores * nch * 128, maxd), 2**31, np.int64)
        E[ks, col] = ss
        return E, counts

    Emat1, counts1 = build_emat(src1, dst1)
    Emat2, counts2 = build_emat(np.asarray(edge_index[0], np.int64),
                                np.asarray(edge_index[1], np.int64))

    def plan_chunk(E, degv):
        valid = E < 2**31
        A = np.zeros(W, np.int64)
        B = np.zeros(W, np.int64)
        dmax = int(degv.max())
        for q in range(W - 1):
            A[q] = int(((E < bases[q + 1]) & valid).sum(axis=1).max())
            B[q] = int(((E >= bases[q] + SPAN) & valid).sum(axis=1).max())
        A[W - 1] = dmax
        K = int(max(dmax, (A + B).max(), 1))
        L = E.shape[0]
        maxd = E.shape[1]
        while True:
            P = np.maximum.accumulate(np.minimum(np.maximum(A, 0), K - B))
            P[W - 1] = K
            n = np.diff(np.concatenate([[0], P]))
            qcls = np.repeat(np.arange(W), n)
            ptr = np.zeros(L, np.int64)
            slotidx = np.zeros((L, K), np.int32)
            slotmask = np.zeros((L, K), bool)
            ok = True
            for t in range(K):
                b = bases[qcls[t]]
                cur = E[np.arange(L), np.minimum(ptr, maxd - 1)]
                vv = ptr < degv
                if np.any(vv & (cur < b)):
                    ok = False
                    break
                fit = vv & (cur >= b) & (cur < b + SPAN)
                slotidx[:, t] = np.where(fit, cur - b, 0)
                slotmask[:, t] = fit
                ptr += fit
            if ok and np.all(ptr == degv):
                return K, qcls, slotidx, slotmask
            K += 1
            assert K < dmax + 24, "edge window planning failed to converge"

    Ks, toff, calls = [], [], []
    blocks_idx, blocks_mask = [], []
    K2s, toff2 = [], []
    blocks_idx2, blocks_me, blocks_mo = [], [], []
    off = 0
    off2 = 0
    for c in range(nch):
        lanes = ((np.arange(n_cores)[:, None] * nch + c) * 128
                 + np.arange(128)[None, :]).ravel()
        # ---- L1 (windowed, self-loops in-plan) ----
        K, qcls, si, sm = plan_chunk(Emat1[lanes], counts1[lanes])
        Ks.append(K)
        toff.append(off)
        cc = []
        t0 = 0
        while t0 < K:
            q = qcls[t0]
            t1 = t0
            nxt_slab = (t0 // SLAB + 1) * SLAB
            while (t1 < K and qcls[t1] == q and t1 - t0 < GMAX
                   and t1 < nxt_slab):
                t1 += 1
            cc.append((t0, t1, int(q)))
            t0 = t1
        calls.append(cc)
        blocks_idx.append(_wrap_idx(si.reshape(n_cores, 128, K), n_cores, K))
        blocks_mask.append(sm.reshape(n_cores, 128, K))
        off += K
        # ---- L2 (paired rows, single window, no self-loops) ----
        E2 = Emat2[lanes]
        degv2 = counts2[lanes]
        K2 = max(int(degv2.max()), 1)
        sub = E2[:, :K2]
        valid = np.arange(K2)[None, :] < degv2[:, None]
        idx2 = np.where(valid, sub >> 1, 0).astype(np.int32)
        par = np.where(valid, sub & 1, 0)
        me = (valid & (par == 0)).astype(np.float32)
        mo = (valid & (par == 1)).astype(np.float32)
        K2s.append(K2)
        toff2.append(off2)
        blocks_idx2.append(_wrap_idx(idx2.reshape(n_cores, 128, K2),
                                     n_cores, K2))
        blocks_me.append(me.reshape(n_cores, 128, K2))
        blocks_mo.append(mo.reshape(n_cores, 128, K2))
        off2 += K2
    TOT, TOT2 = off, off2
    idx16 = np.tile(np.concatenate(blocks_idx, axis=2).astype(np.int16),
                    (1, 8, 1))
    mask = np.concatenate(blocks_mask, axis=2).astype(np.float32)
    idx16_2 = np.tile(np.concatenate(blocks_idx2, axis=2).astype(np.int16),
                      (1, 8, 1))
    me_all = np.concatenate(blocks_me, axis=2)
    mo_all = np.concatenate(blocks_mo, axis=2)
    return (pos, bases, Ks, toff, TOT, calls, idx16, mask,
            K2s, toff2, TOT2, idx16_2, me_all, mo_all)


def _host_prep(x, edge_index, W1, att1_src, att1_dst, W2, att2_src, att2_dst):
    N, F = x.shape
    H, C = att1_src.shape
    HC = H * C
    NCLS = W2.shape[1]
    n_cores = N_CORES
    nch = -(-N // (n_cores * 128))
    npcp = nch * 128
    R = n_cores * npcp

    (pos, bases, Ks, toff, TOT, calls, idx16, mask,
     K2s, toff2, TOT2, idx16_2, me_all, mo_all) = _edge_plan(
        edge_index, N, n_cores, nch, npcp)

    # (c,h) feature permutation: new col c*H+h <- old col h*C+c
    jj = np.arange(HC)
    perm = (jj % H) * C + jj // H
    W1p = W1[:, perm]

    # Folded attention-logit weight columns (independent of column order)
    Wa_s = np.einsum("fhc,hc->fh", W1.reshape(F, H, C), att1_src).astype(np.float32)
    Wa_d = np.einsum("fhc,hc->fh", W1.reshape(F, H, C), att1_dst).astype(np.float32)
    W1e = np.ascontiguousarray(
        np.concatenate([W1p, Wa_s, Wa_d], axis=1), dtype=np.float32)  # [F, 528]

    w2s = (W2 @ att2_src[0]).astype(np.float32)
    w2d = (W2 @ att2_dst[0]).astype(np.float32)
    W2e_flat = np.zeros((HC, 64), np.float32)
    W2e_flat[:, :NCLS] = W2[perm]
    W2e_flat[:, NCLS] = w2s[perm]
    W2e_flat[:, NCLS + 1] = w2d[perm]
    nslab = HC // 128
    W2e = np.ascontiguousarray(
        W2e_flat.reshape(nslab, 128, 64).transpose(1, 0, 2))  # [128, 4, 64]

    import ml_dtypes
    bf = ml_dtypes.bfloat16
    xtab = np.zeros((R, F), np.float32)
    xtab[pos[np.arange(N)]] = x
    xTp = np.ascontiguousarray(xtab.T).astype(bf)   # [F, R] permuted cols
    W1e = W1e.astype(bf)
    ident = np.eye(128, dtype=np.float32).astype(bf)

    cfg = dict(
        N=N, F=F, H=H, C=C, HC=HC, NCLS=NCLS, n_cores=n_cores,
        nch=nch, npcp=npcp, R=R, nslab=nslab,
        Ks=Ks, toff=toff, TOT=TOT, calls=calls, bases=bases, pos=pos,
        K2s=K2s, toff2=toff2, TOT2=TOT2,
        swdge_queues=4, p0_bufs=4,
        gt_bufs=6, g2_bufs=4,
    )
    shared = dict(xTp=xTp, W1e=W1e, W2e=W2e.astype(bf), ident=ident)
    per_core = [
        dict(g1idx=idx16[k], mask=mask[k].astype(bf),
             g2idx=idx16_2[k], me=me_all[k], mo=mo_all[k])
        for k in range(n_cores)
    ]
    return cfg, shared, per_core


# ----------------------------------------------------------------------------
# Device program
# ----------------------------------------------------------------------------

def _build_program(cfg):
    F, HC, NCLS = cfg["F"], cfg["HC"], cfg["NCLS"]
    n_cores, npcp, R = cfg["n_cores"], cfg["npcp"], cfg["R"]
    nslab, TOT, TOT2 = cfg["nslab"], cfg["TOT"], cfg["TOT2"]

    nc = bacc.Bacc("TRN2", target_bir_lowering=False, debug=False,
                   num_devices=n_cores,
                   num_swdge_queues=cfg.get("swdge_queues", 1))

    xTp = nc.dram_tensor("xTp", [F, R], BF16, kind="ExternalInput").ap()
    W1e = nc.dram_tensor("W1e", [F, HC + 16], BF16, kind="ExternalInput").ap()
    W2e = nc.dram_tensor("W2e", [128, nslab, 64], BF16, kind="ExternalInput").ap()
    ident_d = nc.dram_tensor("ident", [128, 128], BF16, kind="ExternalInput").ap()
    g1idx = nc.dram_tensor("g1idx", [128, 8 * TOT], I16,
                           kind="ExternalInput").ap()
    mask_d = nc.dram_tensor("mask", [128, TOT], BF16, kind="ExternalInput").ap()
    g2idx = nc.dram_tensor("g2idx", [128, 8 * TOT2], I16,
                           kind="ExternalInput").ap()
    me_d = nc.dram_tensor("me", [128, TOT2], F32, kind="ExternalInput").ap()
    mo_d = nc.dram_tensor("mo", [128, TOT2], F32, kind="ExternalInput").ap()

    T1 = nc.dram_tensor("T1", [R, 640], BF16).ap()
    tb2_own = nc.dram_tensor("tb2_own", [npcp, 64], BF16).ap()
    tb2_full = nc.dram_tensor("tb2_full", [R, 64], BF16,
                              addr_space="Shared").ap()
    out2 = nc.dram_tensor("out2", [npcp, NCLS], F32, kind="ExternalOutput").ap()

    tensors = dict(xTp=xTp, W1e=W1e, W2e=W2e, ident=ident_d, g1idx=g1idx,
                   mask=mask_d, g2idx=g2idx, me=me_d, mo=mo_d,
                   T1=T1, tb2_own=tb2_own, tb2_full=tb2_full, out2=out2)
    repeat = cfg.get("repeat", 1)
    with tile.TileContext(nc) as tc:
        for _ in range(repeat):
            _emit(tc, cfg, tensors)
    nc.compile()
    return nc


def _emit(tc, cfg, t):
    nc = tc.nc
    H, HC, NCLS = cfg["H"], cfg["HC"], cfg["NCLS"]
    n_cores, nch, npcp, R = cfg["n_cores"], cfg["nch"], cfg["npcp"], cfg["R"]
    nslab = cfg["nslab"]
    NTB = R // 128

    with tc.tile_pool(name="consts", bufs=1) as cpool:
        W1e_sb = cpool.tile([128, HC + 16], BF16)
        nc.sync.dma_start(W1e_sb[:], t["W1e"][:, :])
        W2e_sb = cpool.tile([128, nslab, 64], BF16)
        nc.sync.dma_start(W2e_sb[:], t["W2e"][:, :, :])
        ident_bf = cpool.tile([128, 128], BF16)
        nc.sync.dma_start(ident_bf[:], t["ident"][:, :])
        ald1_all = cpool.tile([128, NTB, H], F32)
        ald1_sb = cpool.tile([128, nch, H], F32)
        tb2self = cpool.tile([128, nch, 64], BF16)

        # ---------------- Phase 0: permuted node table T1 ----------------
        with (
            nc.named_scope("p0"),
            tc.tile_pool(name="p0", bufs=cfg.get("p0_bufs", 4)) as pool,
            tc.tile_pool(name="p0ps", bufs=cfg.get("p0_bufs", 4),
                         space="PSUM") as pps,
        ):
            nblk = R // 512
            for i in range(nblk):
                xt = pool.tile([128, 512], BF16, tag="xt")
                nc.sync.dma_start(xt[:], t["xTp"][:, 512 * i: 512 * i + 512])
                rowB = pool.tile([128, 4, HC + 16], BF16, tag="rowB")
                for j in range(4):
                    # two matmuls: a 512-col PSUM write must stay in one bank
                    ps = pps.tile([128, HC + 16], F32, tag="ps")
                    nc.tensor.matmul(ps[:, 0:HC],
                                     lhsT=xt[:, 128 * j: 128 * j + 128],
                                     rhs=W1e_sb[:, 0:HC], start=True, stop=True)
                    nc.tensor.matmul(ps[:, HC: HC + 16],
                                     lhsT=xt[:, 128 * j: 128 * j + 128],
                                     rhs=W1e_sb[:, HC: HC + 16],
                                     start=True, stop=True)
                    if j < 3:
                        nc.vector.tensor_copy(rowB[:, j, 0:HC], ps[:, 0:HC])
                    else:
                        nc.scalar.copy(rowB[:, j, 0:HC], ps[:, 0:HC])
                    nc.vector.tensor_copy(
                        rowB[:, j, HC: HC + 16].bitcast(F32),
                        ps[:, HC: HC + H])
                    nc.scalar.copy(ald1_all[:, 4 * i + j, :],
                                   ps[:, HC + H: HC + 2 * H])
                # ACT HWDGE ring: xt prefetches must not queue behind the
                # rowB-ready waits of T1 writes on the SP ring
                nc.scalar.dma_start(
                    t["T1"][512 * i: 512 * i + 512, 0: HC + 16].rearrange(
                        "(j p) c -> p j c", p=128),
                    rowB[:],
                )

        pid = nc.partition_id()
        nc.sync.dma_start(ald1_sb[:], ald1_all[:, bass.ds(pid * nch, nch), :])

        if cfg.get("phases", "full") == "p0":
            return
        # ---------------- L1 edge phase ----------------
        with nc.named_scope("l1"):
            _l1_phase(tc, cfg, t, W2e_sb, ident_bf, ald1_sb, tb2self)

        if cfg.get("phases", "full") == "p0+l1":
            return
        # ---------------- allgather ----------------
        with nc.named_scope("ag"):
            if cfg.get("no_collective"):
                nc.sync.dma_start(t["tb2_full"][0:npcp, :], t["tb2_own"][:, :])
            else:
                nc.gpsimd.collective_compute(
                    "AllGather",
                    OP.bypass,
                    replica_groups=[list(range(n_cores))],
                    ins=[t["tb2_own"][:, :]],
                    outs=[t["tb2_full"][:, :]],
                )

        if cfg.get("phases", "full") == "p0+l1+ag":
            return
        # ---------------- L2 edge phase ----------------
        with nc.named_scope("l2"):
            _l2_phase(tc, cfg, t, tb2self)


def _l1_phase(tc, cfg, t, W2e_sb, ident_bf, ald1_sb, tb2self):
    nc = tc.nc
    nch, H, C, HC, NCLS = cfg["nch"], cfg["H"], cfg["C"], cfg["HC"], cfg["NCLS"]
    nslab = cfg["nslab"]
    Ks, toff, calls, bases = cfg["Ks"], cfg["toff"], cfg["calls"], cfg["bases"]
    TOT = cfg["TOT"]
    nq = cfg.get("swdge_queues", 1)

    with (
        tc.tile_pool(name="gt1", bufs=cfg.get("gt_bufs", 2)) as gpool,
        tc.tile_pool(name="meta1", bufs=1) as mpool,
        tc.tile_pool(name="small1", bufs=3) as smpool,
        tc.tile_pool(name="out1", bufs=2) as opool,
        tc.tile_pool(name="ps_u1", bufs=2, space="PSUM") as pp_u,
        tc.tile_pool(name="ps_tr1", bufs=2, space="PSUM") as pp_tr,
        tc.tile_pool(name="ps_o1", bufs=2, space="PSUM") as pp_o,
        tc.tile_pool(name="ps_a1", bufs=2, space="PSUM") as pp_a,
    ):
        idx_all = mpool.tile([128, 8 * TOT], I16, tag="idxall")
        nc.sync.dma_start(idx_all[:], t["g1idx"][:, :])
        msk_all = mpool.tile([128, TOT], BF16, tag="mskall")
        nc.sync.dma_start(msk_all[:], t["mask"][:, :])
        for c in range(nch):
            K = Ks[c]
            off = toff[c]
            stop = cfg.get("l1_stop")
            p_bf = smpool.tile([128, K, H], BF16, tag="pbf")
            ps_u = pp_u.tile([128, HC], F32, tag="u")
            # slab rounds: small gather tiles (deep pipelining) and short
            # DVE-lock pieces so SWDGE descriptor generation stays fed
            for r0 in range(0, K, SLAB):
                r1 = min(r0 + SLAB, K)
                kr = r1 - r0
                gt = gpool.tile([128, SLAB, 640], BF16, tag="gt")
                for (b0, b1, q) in calls[c]:
                    if b0 < r0 or b0 >= r1:
                        continue
                    nk = b1 - b0
                    nc.gpsimd.dma_gather(
                        gt[:, b0 - r0: b1 - r0, :],
                        t["T1"][bases[q]: bases[q] + SPAN, :],
                        idx_all[:, 8 * (off + b0): 8 * (off + b1)],
                        nk * 128, nk * 128, 640,
                        queue_num=_QCTR[0] % nq,
                    )
                    _QCTR[0] += 1
                if stop == "gather":
                    continue
                # p = exp(leakyrelu(al_src[src] + al_dst[dst])) * mask
                s_t = smpool.tile([128, SLAB, H], F32, tag="s")
                nc.vector.tensor_tensor(
                    s_t[:, 0:kr, :],
                    gt[:, 0:kr, HC: HC + 16].bitcast(F32),
                    ald1_sb[:, c, None, :].to_broadcast([128, kr, H]),
                    op=OP.add,
                )
                l_t = smpool.tile([128, SLAB, H], F32, tag="l")
                nc.vector.scalar_tensor_tensor(
                    l_t[:, 0:kr, :], s_t[:, 0:kr, :], 0.2, s_t[:, 0:kr, :],
                    op0=OP.mult, op1=OP.max
                )
                p_t = smpool.tile([128, SLAB, H], F32, tag="p")
                nc.scalar.activation(p_t[:, 0:kr, :], l_t[:, 0:kr, :], AF.Exp)
                nc.vector.tensor_tensor(
                    p_bf[:, r0:r1, :], p_t[:, 0:kr, :],
                    msk_all[:, off + r0: off + r1, None].to_broadcast(
                        [128, kr, H]),
                    op=OP.mult,
                )
                if stop in ("logits", "mult", "agg"):
                    continue
                # in-place alpha multiply, (c,h) layout -> unit inner stride.
                # alpha staged into PSUM via ACT: the TT then holds only one
                # SBUF read port, so SWDGE descriptor-gen is not locked out.
                if cfg.get("alpha_psum", False):
                    pa = pp_a.tile([128, SLAB, H], BF16, tag="pa")
                    nc.scalar.copy(pa[:, 0:kr, :], p_bf[:, r0:r1, :])
                    alpha_src = pa[:, 0:kr, None, :]
                else:
                    alpha_src = p_bf[:, r0:r1, None, :]
                nc.vector.tensor_tensor(
                    gt[:, 0:kr, 0:HC].rearrange("p k (c h) -> p k c h", h=H),
                    gt[:, 0:kr, 0:HC].rearrange("p k (c h) -> p k c h", h=H),
                    alpha_src.to_broadcast([128, kr, C, H]),
                    op=OP.mult,
                )
                for k in range(kr):
                    nc.tensor.matmul(
                        ps_u[:], lhsT=ident_bf[:], rhs=gt[:, k, 0:HC],
                        start=(r0 + k == 0), stop=(r0 + k == K - 1),
                    )
            if stop in ("gather", "logits", "mult", "agg"):
                continue
            zr = smpool.tile([128, H], F32, tag="zr")
            nc.vector.tensor_reduce(
                zr[:], p_bf[:].rearrange("p k h -> p h k"),
                axis=AX.X, op=OP.add,
            )
            zb = smpool.tile([128, H], F32, tag="zb")
            nc.vector.tensor_scalar_max(zb[:], zr[:], 1e-30)
            rz = smpool.tile([128, H], F32, tag="rz")
            nc.vector.reciprocal(rz[:], zb[:])
            h2 = opool.tile([128, HC], F32, tag="h2")
            nc.vector.tensor_tensor(
                h2[:].rearrange("p (c h) -> p c h", h=H),
                ps_u[:].rearrange("p (c h) -> p c h", h=H),
                rz[:, None, :].to_broadcast([128, C, H]),
                op=OP.mult,
            )
            h2r = opool.tile([128, HC], BF16, tag="h2r")
            nc.scalar.activation(h2r[:], h2[:], AF.Relu)
            if stop == "h2":
                continue
            ps_o = pp_o.tile([128, 64], F32, tag="o")
            for j in range(nslab):
                ps_tr = pp_tr.tile([128, 128], BF16, tag="tr")
                nc.tensor.transpose(
                    ps_tr[:], h2r[:, 128 * j: 128 * (j + 1)], ident_bf[:]
                )
                h2t = smpool.tile([128, 128], BF16, tag="h2t")
                nc.scalar.copy(h2t[:], ps_tr[:])
                nc.tensor.matmul(
                    ps_o[:], lhsT=h2t[:], rhs=W2e_sb[:, j, :],
                    start=(j == 0), stop=(j == nslab - 1),
                )
            # tb2 row: [40 cls bf16 | al2_src f32 pair | al2_dst f32 pair | pad]
            nc.vector.tensor_copy(tb2self[:, c, 0:NCLS], ps_o[:, 0:NCLS])
            nc.scalar.copy(tb2self[:, c, NCLS: NCLS + 4].bitcast(F32),
                           ps_o[:, NCLS: NCLS + 2])
            nc.sync.dma_start(t["tb2_own"][128 * c: 128 * (c + 1), :],
                              tb2self[:, c, :])


def _l2_phase(tc, cfg, t, tb2self):
    nc = tc.nc
    nch, NCLS = cfg["nch"], cfg["NCLS"]
    K2s, toff2, TOT2 = cfg["K2s"], cfg["toff2"], cfg["TOT2"]
    nq = cfg.get("swdge_queues", 1)

    with (
        tc.tile_pool(name="gt2", bufs=cfg.get("g2_bufs", 3)) as gpool,
        tc.tile_pool(name="meta2", bufs=1) as mpool,
        tc.tile_pool(name="small2", bufs=3) as smpool,
        tc.tile_pool(name="out2p", bufs=2) as opool,
    ):
        idx_all = mpool.tile([128, 8 * TOT2], I16, tag="idx2all")
        nc.sync.dma_start(idx_all[:], t["g2idx"][:, :])
        me_all = mpool.tile([128, TOT2], F32, tag="meall")
        nc.sync.dma_start(me_all[:], t["me"][:, :])
        mo_all = mpool.tile([128, TOT2], F32, tag="moall")
        nc.sync.dma_start(mo_all[:], t["mo"][:, :])
        tb2p = t["tb2_full"].rearrange("(a b) c -> a (b c)", b=2)
        for c in range(nch):
            K = K2s[c]
            off = toff2[c]
            me = me_all[:, off: off + K]
            mo = mo_all[:, off: off + K]
            gt = gpool.tile([128, K, 128], BF16, tag="gt2")
            for b0 in range(0, K, GMAX):
                b1 = min(b0 + GMAX, K)
                nk = b1 - b0
                nc.gpsimd.dma_gather(
                    gt[:, b0:b1, :], tb2p[:, :],
                    idx_all[:, 8 * (off + b0): 8 * (off + b1)],
                    nk * 128, nk * 128, 128,
                    queue_num=_QCTR[0] % nq,
                )
                _QCTR[0] += 1
            # logits: select even/odd al2_src, add own al2_dst
            ald = tb2self[:, c, NCLS + 2: NCLS + 4].bitcast(F32)  # [128,1]
            v1 = smpool.tile([128, K], F32, tag="v1")
            nc.vector.tensor_tensor(
                v1[:], gt[:, :, NCLS: NCLS + 2].bitcast(F32)[:, :, 0], me,
                op=OP.mult)
            v2 = smpool.tile([128, K], F32, tag="v2")
            nc.vector.tensor_tensor(
                v2[:], gt[:, :, 64 + NCLS: 64 + NCLS + 2].bitcast(F32)[:, :, 0],
                mo, op=OP.mult)
            s2 = smpool.tile([128, K], F32, tag="s2")
            nc.vector.tensor_tensor(s2[:], v1[:], v2[:], op=OP.add)
            s2b = smpool.tile([128, K], F32, tag="s2b")
            nc.vector.tensor_tensor(
                s2b[:], s2[:], ald.to_broadcast([128, K]), op=OP.add)
            l2t = smpool.tile([128, K], F32, tag="l2")
            nc.vector.scalar_tensor_tensor(
                l2t[:], s2b[:], 0.2, s2b[:], op0=OP.mult, op1=OP.max)
            p2 = smpool.tile([128, K], F32, tag="p2")
            nc.scalar.activation(p2[:], l2t[:], AF.Exp)
            aE = smpool.tile([128, K], BF16, tag="aE")
            nc.vector.tensor_tensor(aE[:], p2[:], me, op=OP.mult)
            aO = smpool.tile([128, K], BF16, tag="aO")
            nc.vector.tensor_tensor(aO[:], p2[:], mo, op=OP.mult)
            # z from the already-masked weights (1-src reduces, no port lock)
            zE = smpool.tile([128, 1], F32, tag="zE")
            nc.vector.tensor_reduce(zE[:], aE[:], axis=AX.X, op=OP.add)
            zO = smpool.tile([128, 1], F32, tag="zO")
            nc.vector.tensor_reduce(zO[:], aO[:], axis=AX.X, op=OP.add)
            z2 = smpool.tile([128, 1], F32, tag="z2")
            nc.vector.tensor_tensor(z2[:], zE[:], zO[:], op=OP.add)
            # self-loop: logit from own al2_src + own al2_dst
            ss = smpool.tile([128, 1], F32, tag="ss")
            nc.vector.tensor_tensor(
                ss[:], tb2self[:, c, NCLS: NCLS + 2].bitcast(F32), ald,
                op=OP.add)
            lss = smpool.tile([128, 1], F32, tag="lss")
            nc.vector.scalar_tensor_tensor(
                lss[:], ss[:], 0.2, ss[:], op0=OP.mult, op1=OP.max)
            p2s = smpool.tile([128, 1], F32, tag="p2s")
            nc.scalar.activation(p2s[:], lss[:], AF.Exp)
            z2b = smpool.tile([128, 1], F32, tag="z2b")
            nc.vector.tensor_tensor(z2b[:], z2[:], p2s[:], op=OP.add)
            zc = smpool.tile([128, 1], F32, tag="zc")
            nc.vector.tensor_scalar_max(zc[:], z2b[:], 1e-30)
            rz2 = smpool.tile([128, 1], F32, tag="rz2")
            nc.vector.reciprocal(rz2[:], zc[:])
            # weighted messages in place, then reduce over slots
            nc.vector.tensor_tensor(
                gt[:, :, 0:NCLS], gt[:, :, 0:NCLS],
                aE[:, :, None].to_broadcast([128, K, NCLS]), op=OP.mult)
            nc.vector.tensor_tensor(
                gt[:, :, 64: 64 + NCLS], gt[:, :, 64: 64 + NCLS],
                aO[:, :, None].to_broadcast([128, K, NCLS]), op=OP.mult)
            u2a = smpool.tile([128, NCLS], F32, tag="u2a")
            nc.vector.tensor_reduce(
                u2a[:], gt[:, :, 0:NCLS].rearrange("p k f -> p f k"),
                axis=AX.X, op=OP.add)
            u2b = smpool.tile([128, NCLS], F32, tag="u2b")
            nc.vector.tensor_reduce(
                u2b[:], gt[:, :, 64: 64 + NCLS].rearrange("p k f -> p f k"),
                axis=AX.X, op=OP.add)
            u2 = smpool.tile([128, NCLS], F32, tag="u2")
            nc.vector.tensor_tensor(u2[:], u2a[:], u2b[:], op=OP.add)
            msel = smpool.tile([128, NCLS], F32, tag="msel")
            nc.vector.tensor_tensor(
                msel[:], tb2self[:, c, 0:NCLS],
                p2s[:].to_broadcast([128, NCLS]), op=OP.mult)
            u2c = smpool.tile([128, NCLS], F32, tag="u2c")
            nc.vector.tensor_tensor(u2c[:], u2[:], msel[:], op=OP.add)
            o2 = opool.tile([128, NCLS], F32, tag="o2")
            nc.vector.tensor_tensor(
                o2[:], u2c[:], rz2[:].to_broadcast([128, NCLS]), op=OP.mult)
            nc.sync.dma_start(t["out2"][128 * c: 128 * (c + 1), :], o2[:])


# ----------------------------------------------------------------------------
# PJRT execution (with on-device iteration chaining for timing)
# ----------------------------------------------------------------------------

def _pjrt_exec(nc, in_maps, n_cores, iters=1, reps=3):
    import jax
    import numpy as _np
    from jax.sharding import Mesh, PartitionSpec
    from jax.experimental.shard_map import shard_map
    from concourse import bass2jax as b2j
    from concourse import mybir as _mb

    b2j.install_neuronx_cc_hook()
    partition_name = (nc.partition_id_tensor.name
                      if nc.partition_id_tensor else None)
    in_names, out_names, out_avals, zero_outs = [], [], [], []
    for alloc in nc.m.functions[0].allocations:
        if not isinstance(alloc, _mb.MemoryLocationSet):
            continue
        name = alloc.memorylocations[0].name
        if alloc.kind == "ExternalInput":
            if name != partition_name:
                in_names.append(name)
        elif alloc.kind == "ExternalOutput":
            shape = tuple(alloc.tensor_shape)
            dtype = _mb.dt.np(alloc.dtype)
            out_names.append(name)
            out_avals.append(jax.core.ShapedArray(shape, dtype))
            zero_outs.append(_np.zeros(shape, dtype))
    n_params = len(in_names)
    all_in_names = in_names + out_names
    if partition_name is not None:
        all_in_names = all_in_names + [partition_name]

    def _body(*args):
        ins = list(args[:n_params])
        zo = list(args[n_params:])
        for _ in range(iters):
            operands = ins + zo
            if partition_name is not None:
                operands.append(b2j.partition_id_tensor())
            outs = _bass_exec_bind(b2j, operands, out_avals, all_in_names,
                                   out_names, nc)
            zo = list(outs)
        return tuple(zo)

    devices = jax.devices()[:n_cores]
    mesh = Mesh(_np.asarray(devices), ("core",))
    in_specs = (PartitionSpec("core"),) * (n_params + len(out_names))
    out_specs = (PartitionSpec("core"),) * len(out_names)
    sharded = jax.jit(shard_map(_body, mesh=mesh, in_specs=in_specs,
                                out_specs=out_specs, check_rep=False),
                      keep_unused=True)
    concat_in = [
        _np.concatenate([_np.asarray(in_maps[c][nm]) for c in range(n_cores)],
                        axis=0)
        for nm in in_names
    ]
    concat_zeros = [_np.zeros((n_cores * z.shape[0], *z.shape[1:]), z.dtype)
                    for z in zero_outs]
    import time as _time
    from jax.sharding import NamedSharding
    sh = NamedSharding(mesh, PartitionSpec("core"))
    dev_in = [jax.device_put(a, sh) for a in concat_in]
    dev_zeros = [jax.device_put(a, sh) for a in concat_zeros]
    jax.block_until_ready(dev_in + dev_zeros)
    out_arrs = sharded(*dev_in, *dev_zeros)
    jax.block_until_ready(out_arrs)
    times = []
    for _ in range(reps):
        t0 = _time.perf_counter()
        out_arrs = sharded(*dev_in, *dev_zeros)
        jax.block_until_ready(out_arrs)
        times.append(_time.perf_counter() - t0)
    dt = min(times)
    results = [
        {nm: _np.asarray(out_arrs[i]).reshape(n_cores, *out_avals[i].shape)[c]
         for i, nm in enumerate(out_names)}
        for c in range(n_cores)
    ]
    return results, dt


def _bass_exec_bind(b2j, operands, out_avals, in_names, out_names, nc):
    return b2j._bass_exec_p.bind(
        *operands,
        out_avals=tuple(out_avals),
        in_names=tuple(in_names),
        out_names=tuple(out_names),
        lowering_input_output_aliases=(),
        sim_require_finite=True,
        sim_require_nnan=True,
        nc=nc,
    )


# ----------------------------------------------------------------------------
# Entry point
# ----------------------------------------------------------------------------

_CACHE = {}


def _run(inputs, trace=False):
    x = np.asarray(inputs["x"], np.float32)
    edge_index = np.asarray(inputs["edge_index"], np.int32)
    W1 = np.asarray(inputs["W1"], np.float32)
    a1s = np.asarray(inputs["att1_src"], np.float32)
    a1d = np.asarray(inputs["att1_dst"], np.float32)
    W2 = np.asarray(inputs["W2"], np.float32)
    a2s = np.asarray(inputs["att2_src"], np.float32)
    a2d = np.asarray(inputs["att2_dst"], np.float32)
    b1 = np.asarray(inputs["b1"], np.float32)
    b2 = np.asarray(inputs["b2"], np.float32)
    assert not b1.any() and not b2.any(), "nonzero bias unsupported"

    key = hashlib.sha1(
        b"v3" + edge_index.tobytes() + np.int64(x.shape).tobytes()
    ).hexdigest()
    cfg, shared, per_core = _host_prep(x, edge_index, W1, a1s, a1d, W2, a2s, a2d)
    if key not in _CACHE:
        _CACHE[key] = _build_program(cfg)
    nc = _CACHE[key]

    in_maps = []
    for k in range(cfg["n_cores"]):
        m = dict(shared)
        m.update(per_core[k])
        in_maps.append(m)
    res = run_bass_kernel_spmd(nc, in_maps, list(range(cfg["n_cores"])),
                               trace=trace)
    out = gather_out([res.results[k]["out2"] for k in range(cfg["n_cores"])],
                     cfg)
    return out.astype(np.float32), res


def gather_out(outs, cfg):
    allrows = np.concatenate(outs, axis=0)          # [R, NCLS] permuted
    return allrows[cfg["pos"][: cfg["N"]]]


def kernel(**inputs):
    out, _ = _run(inputs, trace=False)
    return out


# revision 32
# speedup vs baseline: 2.3542x; 1.0321x over previous
"""GAT (2-layer, PyG-default) Trainium2 Bass kernel, 8-core SPMD.

v3 — trace-driven rework of the dst-major design (baseline 2.0ms ->
~1.56ms).  Measured constraints that shaped it: every gathered row is
one SWDGE descriptor costing ~70-90ns of SDMA-engine time regardless
of size (the kernel is descriptor-count-bound, ~230k descs/core), a
dma_gather call with >1024 indices wedges the device, calls with >512
descriptors block the GpSimd engine until ring space frees, and any
2-input DVE op holds the SBUF port pair that SWDGE descriptor
generation needs (DVE TENSOR_TENSOR time stalls the gather pipe).

  - Node permutation is globally degree-balanced: nodes ranked by
    in-degree are dealt into (chunk, core, lane) so each chunk's 1024
    lanes (128 per core x 8 cores) have near-uniform degree, shrinking
    the per-chunk slot count K toward the mean degree.
  - L1 rows are stored feature-transposed (c,h): the per-edge softmax
    weight broadcast then has unit inner stride on every operand, so
    the big per-round message multiply runs in DVE 2x_1P mode
    (in-place on the gather tile; no per-slot DVE ops).
  - L1 chunks are processed in SLAB-slot rounds: small gather tiles
    give a deep (6-buffer) gather pipeline, gather calls stay at <=4
    slots (512 descriptors, fire-and-forget), and each DVE-lock piece
    is ~3us so the 4 SWDGE queue rings (~9us of buffered descriptors)
    ride through it.  PSUM accumulates across rounds.
  - L2 gathers PAIRS of compact 128B tb2 rows (idx = src>>1, 256B
    descriptors): the pair index range (25088 < 32768) fits one int16
    window, killing L1's 4-window slot inflation for L2.  Even/odd row
    selection is folded into the alpha masks (me/mo).  L2 self-loop
    contributions come from an SBUF-resident tb2self captured while L1
    writes tb2 rows, so they never touch the gather path.
  - Phase 0 splits the PSUM->bf16 casts 3:1 between Vector and Scalar;
    transposes and the W2 projection run in bf16.

Self-contained: only needs numpy + the concourse tree at /opt/trn_rl_repo.
"""

import hashlib
import sys

import numpy as np

for _p in ("/opt/trn_rl_repo",):
    if _p not in sys.path:
        sys.path.insert(0, _p)

import concourse.bacc as bacc
import concourse.bass as bass
import concourse.tile as tile
from concourse import mybir
from concourse.bass_utils import run_bass_kernel_spmd

F32 = mybir.dt.float32
BF16 = mybir.dt.bfloat16
I16 = mybir.dt.int16
AF = mybir.ActivationFunctionType
OP = mybir.AluOpType
AX = mybir.AxisListType

N_CORES = 8
SPAN = 32768
W = 4
GMAX = 4
SLAB = 12  # L1 slots per gather tile / DVE-multiply piece (multiple of GMAX)
_QCTR = [0]  # global SWDGE queue round-robin


# ----------------------------------------------------------------------------
# Host-side edge planning
# ----------------------------------------------------------------------------

def _wrap_idx(si, n_cores, K):
    """[n_cores,128,K] int -> [n_cores,16,8K] in the dma_gather idx layout
    (idx of token T, partition p lands at [p%16, 8*T + p//16])."""
    tmp = si.reshape(n_cores, 8, 16, K)
    return np.ascontiguousarray(tmp.transpose(0, 2, 3, 1)).reshape(
        n_cores, 16, 8 * K)


def _edge_plan(edge_index, N, n_cores, nch, npcp):
    """Degree-balanced dst-major plans for both layers.

    L1: self-loops included, W=4 overlapping 32768-row windows.
    L2: self-loops excluded, rows gathered in PAIRS (idx = srow>>1) from
        a single window, with even/odd masks me/mo.
    """
    R = n_cores * npcp
    bases = [round(q * (R - SPAN) / (W - 1)) for q in range(W)]

    src1 = np.concatenate([np.asarray(edge_index[0], np.int64), np.arange(N)])
    dst1 = np.concatenate([np.asarray(edge_index[1], np.int64), np.arange(N)])
    deg = np.bincount(dst1, minlength=R)
    # Deal degree-ranked nodes into (chunk, core, lane): chunk lanes get
    # near-uniform degree across all cores.
    order = np.argsort(deg, kind="stable")
    blk = 128 * n_cores
    ii = np.arange(R)
    chunk = ii // blk
    core = (ii % blk) // 128
    lane = ii % 128
    pos = np.empty(R, np.int64)
    pos[order] = core * npcp + chunk * 128 + lane

    def build_emat(src, dst):
        srow = pos[src]
        dpos = pos[dst]
        key = dpos // npcp * (nch * 128) + dpos % npcp
        order_e = np.lexsort((srow, key))
        ks, ss = key[order_e], srow[order_e]
        counts = np.bincount(ks, minlength=n_cores * nch * 128)
        maxd = max(int(counts.max()), 1)
        starts = np.zeros(len(counts) + 1, np.int64)
        np.cumsum(counts, out=starts[1:])
        col = np.arange(len(ss)) - starts[ks]
        E = np.full((n_cores * nch * 128, maxd), 2**31, np.int64)
        E[ks, col] = ss
        return E, counts

    Emat1, counts1 = build_emat(src1, dst1)
    Emat2, counts2 = build_emat(np.asarray(edge_index[0], np.int64),
                                np.asarray(edge_index[1], np.int64))

    def plan_chunk(E, degv):
        valid = E < 2**31
        A = np.zeros(W, np.int64)
        B = np.zeros(W, np.int64)
        dmax = int(degv.max())
        for q in range(W - 1):
            A[q] = int(((E < bases[q + 1]) & valid).sum(axis=1).max())
            B[q] = int(((E >= bases[q] + SPAN) & valid).sum(axis=1).max())
        A[W - 1] = dmax
        K = int(max(dmax, (A + B).max(), 1))
        L = E.shape[0]
        maxd = E.shape[1]
        while True:
            P = np.maximum.accumulate(np.minimum(np.maximum(A, 0), K - B))
            P[W - 1] = K
            n = np.diff(np.concatenate([[0], P]))
            qcls = np.repeat(np.arange(W), n)
            ptr = np.zeros(L, np.int64)
            slotidx = np.zeros((L, K), np.int32)
            slotmask = np.zeros((L, K), bool)
            ok = True
            for t in range(K):
                b = bases[qcls[t]]
                cur = E[np.arange(L), np.minimum(ptr, maxd - 1)]
                vv = ptr < degv
                if np.any(vv & (cur < b)):
                    ok = False
                    break
                fit = vv & (cur >= b) & (cur < b + SPAN)
                slotidx[:, t] = np.where(fit, cur - b, 0)
                slotmask[:, t] = fit
                ptr += fit
            if ok and np.all(ptr == degv):
                return K, qcls, slotidx, slotmask
            K += 1
            assert K < dmax + 24, "edge window planning failed to converge"

    Ks, toff, calls = [], [], []
    blocks_idx, blocks_mask = [], []
    K2s, toff2 = [], []
    blocks_idx2, blocks_me, blocks_mo = [], [], []
    off = 0
    off2 = 0
    for c in range(nch):
        lanes = ((np.arange(n_cores)[:, None] * nch + c) * 128
                 + np.arange(128)[None, :]).ravel()
        # ---- L1 (windowed, self-loops in-plan) ----
        K, qcls, si, sm = plan_chunk(Emat1[lanes], counts1[lanes])
        Ks.append(K)
        toff.append(off)
        cc = []
        t0 = 0
        while t0 < K:
            q = qcls[t0]
            t1 = t0
            nxt_slab = (t0 // SLAB + 1) * SLAB
            while (t1 < K and qcls[t1] == q and t1 - t0 < GMAX
                   and t1 < nxt_slab):
                t1 += 1
            cc.append((t0, t1, int(q)))
            t0 = t1
        calls.append(cc)
        blocks_idx.append(_wrap_idx(si.reshape(n_cores, 128, K), n_cores, K))
        blocks_mask.append(sm.reshape(n_cores, 128, K))
        off += K
        # ---- L2 (paired rows, single window, no self-loops) ----
        E2 = Emat2[lanes]
        degv2 = counts2[lanes]
        K2 = max(int(degv2.max()), 1)
        sub = E2[:, :K2]
        valid = np.arange(K2)[None, :] < degv2[:, None]
        idx2 = np.where(valid, sub >> 1, 0).astype(np.int32)
        par = np.where(valid, sub & 1, 0)
        me = (valid & (par == 0)).astype(np.float32)
        mo = (valid & (par == 1)).astype(np.float32)
        K2s.append(K2)
        toff2.append(off2)
        blocks_idx2.append(_wrap_idx(idx2.reshape(n_cores, 128, K2),
                                     n_cores, K2))
        blocks_me.append(me.reshape(n_cores, 128, K2))
        blocks_mo.append(mo.reshape(n_cores, 128, K2))
        off2 += K2
    TOT, TOT2 = off, off2
    idx16 = np.tile(np.concatenate(blocks_idx, axis=2).astype(np.int16),
                    (1, 8, 1))
    mask = np.concatenate(blocks_mask, axis=2).astype(np.float32)
    idx16_2 = np.tile(np.concatenate(blocks_idx2, axis=2).astype(np.int16),
                      (1, 8, 1))
    me_all = np.concatenate(blocks_me, axis=2)
    mo_all = np.concatenate(blocks_mo, axis=2)
    return (pos, bases, Ks, toff, TOT, calls, idx16, mask,
            K2s, toff2, TOT2, idx16_2, me_all, mo_all)


def _host_prep(x, edge_index, W1, att1_src, att1_dst, W2, att2_src, att2_dst):
    N, F = x.shape
    H, C = att1_src.shape
    HC = H * C
    NCLS = W2.shape[1]
    n_cores = N_CORES
    nch = -(-N // (n_cores * 128))
    npcp = nch * 128
    R = n_cores * npcp

    (pos, bases, Ks, toff, TOT, calls, idx16, mask,
     K2s, toff2, TOT2, idx16_2, me_all, mo_all) = _edge_plan(
        edge_index, N, n_cores, nch, npcp)

    # (c,h) feature permutation: new col c*H+h <- old col h*C+c
    jj = np.arange(HC)
    perm = (jj % H) * C + jj // H
    W1p = W1[:, perm]

    # Folded attention-logit weight columns (independent of column order)
    Wa_s = np.einsum("fhc,hc->fh", W1.reshape(F, H, C), att1_src).astype(np.float32)
    Wa_d = np.einsum("fhc,hc->fh", W1.reshape(F, H, C), att1_dst).astype(np.float32)
    W1e = np.ascontiguousarray(
        np.concatenate([W1p, Wa_s, Wa_d], axis=1), dtype=np.float32)  # [F, 528]

    w2s = (W2 @ att2_src[0]).astype(np.float32)
    w2d = (W2 @ att2_dst[0]).astype(np.float32)
    W2e_flat = np.zeros((HC, 64), np.float32)
    W2e_flat[:, :NCLS] = W2[perm]
    W2e_flat[:, NCLS] = w2s[perm]
    W2e_flat[:, NCLS + 1] = w2d[perm]
    nslab = HC // 128
    W2e = np.ascontiguousarray(
        W2e_flat.reshape(nslab, 128, 64).transpose(1, 0, 2))  # [128, 4, 64]

    import ml_dtypes
    bf = ml_dtypes.bfloat16
    xtab = np.zeros((R, F), np.float32)
    xtab[pos[np.arange(N)]] = x
    xTp = np.ascontiguousarray(xtab.T).astype(bf)   # [F, R] permuted cols
    W1e = W1e.astype(bf)
    ident = np.eye(128, dtype=np.float32).astype(bf)

    cfg = dict(
        N=N, F=F, H=H, C=C, HC=HC, NCLS=NCLS, n_cores=n_cores,
        nch=nch, npcp=npcp, R=R, nslab=nslab,
        Ks=Ks, toff=toff, TOT=TOT, calls=calls, bases=bases, pos=pos,
        K2s=K2s, toff2=toff2, TOT2=TOT2,
        swdge_queues=4, p0_bufs=4,
        gt_bufs=6, g2_bufs=4,
    )
    shared = dict(xTp=xTp, W1e=W1e, W2e=W2e.astype(bf), ident=ident)
    per_core = [
        dict(g1idx=idx16[k], mask=mask[k].astype(bf),
             g2idx=idx16_2[k], me=me_all[k], mo=mo_all[k])
        for k in range(n_cores)
    ]
    return cfg, shared, per_core


# ----------------------------------------------------------------------------
# Device program
# ----------------------------------------------------------------------------

def _build_program(cfg):
    F, HC, NCLS = cfg["F"], cfg["HC"], cfg["NCLS"]
    n_cores, npcp, R = cfg["n_cores"], cfg["npcp"], cfg["R"]
    nslab, TOT, TOT2 = cfg["nslab"], cfg["TOT"], cfg["TOT2"]

    nc = bacc.Bacc("TRN2", target_bir_lowering=False, debug=False,
                   num_devices=n_cores,
                   num_swdge_queues=cfg.get("swdge_queues", 1))

    xTp = nc.dram_tensor("xTp", [F, R], BF16, kind="ExternalInput").ap()
    W1e = nc.dram_tensor("W1e", [F, HC + 16], BF16, kind="ExternalInput").ap()
    W2e = nc.dram_tensor("W2e", [128, nslab, 64], BF16, kind="ExternalInput").ap()
    ident_d = nc.dram_tensor("ident", [128, 128], BF16, kind="ExternalInput").ap()
    g1idx = nc.dram_tensor("g1idx", [128, 8 * TOT], I16,
                           kind="ExternalInput").ap()
    mask_d = nc.dram_tensor("mask", [128, TOT], BF16, kind="ExternalInput").ap()
    g2idx = nc.dram_tensor("g2idx", [128, 8 * TOT2], I16,
                           kind="ExternalInput").ap()
    me_d = nc.dram_tensor("me", [128, TOT2], F32, kind="ExternalInput").ap()
    mo_d = nc.dram_tensor("mo", [128, TOT2], F32, kind="ExternalInput").ap()

    T1 = nc.dram_tensor("T1", [R, 640], BF16).ap()
    tb2_own = nc.dram_tensor("tb2_own", [npcp, 64], BF16).ap()
    tb2_full = nc.dram_tensor("tb2_full", [R, 64], BF16,
                              addr_space="Shared").ap()
    out2 = nc.dram_tensor("out2", [npcp, NCLS], F32, kind="ExternalOutput").ap()

    tensors = dict(xTp=xTp, W1e=W1e, W2e=W2e, ident=ident_d, g1idx=g1idx,
                   mask=mask_d, g2idx=g2idx, me=me_d, mo=mo_d,
                   T1=T1, tb2_own=tb2_own, tb2_full=tb2_full, out2=out2)
    repeat = cfg.get("repeat", 1)
    with tile.TileContext(nc) as tc:
        for _ in range(repeat):
            _emit(tc, cfg, tensors)
    nc.compile()
    return nc


def _emit(tc, cfg, t):
    nc = tc.nc
    H, HC, NCLS = cfg["H"], cfg["HC"], cfg["NCLS"]
    n_cores, nch, npcp, R = cfg["n_cores"], cfg["nch"], cfg["npcp"], cfg["R"]
    nslab = cfg["nslab"]
    NTB = R // 128

    with tc.tile_pool(name="consts", bufs=1) as cpool:
        W1e_sb = cpool.tile([128, HC + 16], BF16)
        nc.sync.dma_start(W1e_sb[:], t["W1e"][:, :])
        W2e_sb = cpool.tile([128, nslab, 64], BF16)
        nc.sync.dma_start(W2e_sb[:], t["W2e"][:, :, :])
        ident_bf = cpool.tile([128, 128], BF16)
        nc.sync.dma_start(ident_bf[:], t["ident"][:, :])
        ald1_all = cpool.tile([128, NTB, H], F32)
        ald1_sb = cpool.tile([128, nch, H], F32)
        tb2self = cpool.tile([128, nch, 64], BF16)

        # ---------------- Phase 0: permuted node table T1 ----------------
        with (
            nc.named_scope("p0"),
            tc.tile_pool(name="p0", bufs=cfg.get("p0_bufs", 4)) as pool,
            tc.tile_pool(name="p0ps", bufs=cfg.get("p0_bufs", 4),
                         space="PSUM") as pps,
        ):
            nblk = R // 512
            for i in range(nblk):
                xt = pool.tile([128, 512], BF16, tag="xt")
                nc.sync.dma_start(xt[:], t["xTp"][:, 512 * i: 512 * i + 512])
                rowB = pool.tile([128, 4, HC + 16], BF16, tag="rowB")
                for j in range(4):
                    # two matmuls: a 512-col PSUM write must stay in one bank
                    ps = pps.tile([128, HC + 16], F32, tag="ps")
                    nc.tensor.matmul(ps[:, 0:HC],
                                     lhsT=xt[:, 128 * j: 128 * j + 128],
                                     rhs=W1e_sb[:, 0:HC], start=True, stop=True)
                    nc.tensor.matmul(ps[:, HC: HC + 16],
                                     lhsT=xt[:, 128 * j: 128 * j + 128],
                                     rhs=W1e_sb[:, HC: HC + 16],
                                     start=True, stop=True)
                    if j < 3:
                        nc.vector.tensor_copy(rowB[:, j, 0:HC], ps[:, 0:HC])
                    else:
                        nc.scalar.copy(rowB[:, j, 0:HC], ps[:, 0:HC])
                    nc.vector.tensor_copy(
                        rowB[:, j, HC: HC + 16].bitcast(F32),
                        ps[:, HC: HC + H])
                    nc.scalar.copy(ald1_all[:, 4 * i + j, :],
                                   ps[:, HC + H: HC + 2 * H])
                # ACT HWDGE ring: xt prefetches must not queue behind the
                # rowB-ready waits of T1 writes on the SP ring
                nc.scalar.dma_start(
                    t["T1"][512 * i: 512 * i + 512, 0: HC + 16].rearrange(
                        "(j p) c -> p j c", p=128),
                    rowB[:],
                )

        pid = nc.partition_id()
        nc.sync.dma_start(ald1_sb[:], ald1_all[:, bass.ds(pid * nch, nch), :])

        if cfg.get("phases", "full") == "p0":
            return
        # ---------------- L1 edge phase ----------------
        with nc.named_scope("l1"):
            _l1_phase(tc, cfg, t, W2e_sb, ident_bf, ald1_sb, tb2self)

        if cfg.get("phases", "full") == "p0+l1":
            return
        # ---------------- allgather ----------------
        with nc.named_scope("ag"):
            if cfg.get("no_collective"):
                nc.sync.dma_start(t["tb2_full"][0:npcp, :], t["tb2_own"][:, :])
            else:
                nc.gpsimd.collective_compute(
                    "AllGather",
                    OP.bypass,
                    replica_groups=[list(range(n_cores))],
                    ins=[t["tb2_own"][:, :]],
                    outs=[t["tb2_full"][:, :]],
                )

        if cfg.get("phases", "full") == "p0+l1+ag":
            return
        # ---------------- L2 edge phase ----------------
        with nc.named_scope("l2"):
            _l2_phase(tc, cfg, t, tb2self)


def _l1_phase(tc, cfg, t, W2e_sb, ident_bf, ald1_sb, tb2self):
    nc = tc.nc
    nch, H, C, HC, NCLS = cfg["nch"], cfg["H"], cfg["C"], cfg["HC"], cfg["NCLS"]
    nslab = cfg["nslab"]
    Ks, toff, calls, bases = cfg["Ks"], cfg["toff"], cfg["calls"], cfg["bases"]
    TOT = cfg["TOT"]
    nq = cfg.get("swdge_queues", 1)

    with (
        tc.tile_pool(name="gt1", bufs=cfg.get("gt_bufs", 2)) as gpool,
        tc.tile_pool(name="meta1", bufs=1) as mpool,
        tc.tile_pool(name="small1", bufs=3) as smpool,
        tc.tile_pool(name="out1", bufs=2) as opool,
        tc.tile_pool(name="ps_u1", bufs=2, space="PSUM") as pp_u,
        tc.tile_pool(name="ps_tr1", bufs=2, space="PSUM") as pp_tr,
        tc.tile_pool(name="ps_o1", bufs=2, space="PSUM") as pp_o,
        tc.tile_pool(name="ps_a1", bufs=2, space="PSUM") as pp_a,
    ):
        idx_all = mpool.tile([128, 8 * TOT], I16, tag="idxall")
        nc.sync.dma_start(idx_all[:], t["g1idx"][:, :])
        msk_all = mpool.tile([128, TOT], BF16, tag="mskall")
        nc.sync.dma_start(msk_all[:], t["mask"][:, :])
        for c in range(nch):
            K = Ks[c]
            off = toff[c]
            stop = cfg.get("l1_stop")
            p_bf = smpool.tile([128, K, H], BF16, tag="pbf")
            ps_u = pp_u.tile([128, HC], F32, tag="u")
            # slab rounds: small gather tiles (deep pipelining) and short
            # DVE-lock pieces so SWDGE descriptor generation stays fed
            for r0 in range(0, K, SLAB):
                r1 = min(r0 + SLAB, K)
                kr = r1 - r0
                gt = gpool.tile([128, SLAB, 640], BF16, tag="gt")
                for (b0, b1, q) in calls[c]:
                    if b0 < r0 or b0 >= r1:
                        continue
                    nk = b1 - b0
                    nc.gpsimd.dma_gather(
                        gt[:, b0 - r0: b1 - r0, :],
                        t["T1"][bases[q]: bases[q] + SPAN, :],
                        idx_all[:, 8 * (off + b0): 8 * (off + b1)],
                        nk * 128, nk * 128, 640,
                        queue_num=_QCTR[0] % nq,
                    )
                    _QCTR[0] += 1
                if stop == "gather":
                    continue
                # p = exp(leakyrelu(al_src[src] + al_dst[dst])) * mask
                s_t = smpool.tile([128, SLAB, H], F32, tag="s")
                nc.vector.tensor_tensor(
                    s_t[:, 0:kr, :],
                    gt[:, 0:kr, HC: HC + 16].bitcast(F32),
                    ald1_sb[:, c, None, :].to_broadcast([128, kr, H]),
                    op=OP.add,
                )
                l_t = smpool.tile([128, SLAB, H], F32, tag="l")
                nc.vector.scalar_tensor_tensor(
                    l_t[:, 0:kr, :], s_t[:, 0:kr, :], 0.2, s_t[:, 0:kr, :],
                    op0=OP.mult, op1=OP.max
                )
                p_t = smpool.tile([128, SLAB, H], F32, tag="p")
                nc.scalar.activation(p_t[:, 0:kr, :], l_t[:, 0:kr, :], AF.Exp)
                nc.vector.tensor_tensor(
                    p_bf[:, r0:r1, :], p_t[:, 0:kr, :],
                    msk_all[:, off + r0: off + r1, None].to_broadcast(
                        [128, kr, H]),
                    op=OP.mult,
                )
                if stop in ("logits", "mult", "agg"):
                    continue
                # in-place alpha multiply, (c,h) layout -> unit inner stride.
                # alpha staged into PSUM via ACT: the TT then holds only one
                # SBUF read port, so SWDGE descriptor-gen is not locked out.
                if cfg.get("alpha_psum", False):
                    pa = pp_a.tile([128, SLAB, H], BF16, tag="pa")
                    nc.scalar.copy(pa[:, 0:kr, :], p_bf[:, r0:r1, :])
                    alpha_src = pa[:, 0:kr, None, :]
                else:
                    alpha_src = p_bf[:, r0:r1, None, :]
                nc.vector.tensor_tensor(
                    gt[:, 0:kr, 0:HC].rearrange("p k (c h) -> p k c h", h=H),
                    gt[:, 0:kr, 0:HC].rearrange("p k (c h) -> p k c h", h=H),
                    alpha_src.to_broadcast([128, kr, C, H]),
                    op=OP.mult,
                )
                for k in range(kr):
                    nc.tensor.matmul(
                        ps_u[:], lhsT=ident_bf[:], rhs=gt[:, k, 0:HC],
                        start=(r0 + k == 0), stop=(r0 + k == K - 1),
                    )
            if stop in ("gather", "logits", "mult", "agg"):
                continue
            zr = smpool.tile([128, H], F32, tag="zr")
            nc.vector.tensor_reduce(
                zr[:], p_bf[:].rearrange("p k h -> p h k"),
                axis=AX.X, op=OP.add,
            )
            zb = smpool.tile([128, H], F32, tag="zb")
            nc.vector.tensor_scalar_max(zb[:], zr[:], 1e-30)
            rz = smpool.tile([128, H], F32, tag="rz")
            nc.vector.reciprocal(rz[:], zb[:])
            h2 = opool.tile([128, HC], F32, tag="h2")
            nc.vector.tensor_tensor(
                h2[:].rearrange("p (c h) -> p c h", h=H),
                ps_u[:].rearrange("p (c h) -> p c h", h=H),
                rz[:, None, :].to_broadcast([128, C, H]),
                op=OP.mult,
            )
            h2r = opool.tile([128, HC], BF16, tag="h2r")
            nc.scalar.activation(h2r[:], h2[:], AF.Relu)
            if stop == "h2":
                continue
            ps_o = pp_o.tile([128, 64], F32, tag="o")
            for j in range(nslab):
                ps_tr = pp_tr.tile([128, 128], BF16, tag="tr")
                nc.tensor.transpose(
                    ps_tr[:], h2r[:, 128 * j: 128 * (j + 1)], ident_bf[:]
                )
                h2t = smpool.tile([128, 128], BF16, tag="h2t")
                nc.scalar.copy(h2t[:], ps_tr[:])
                nc.tensor.matmul(
                    ps_o[:], lhsT=h2t[:], rhs=W2e_sb[:, j, :],
                    start=(j == 0), stop=(j == nslab - 1),
                )
            # tb2 row: [40 cls bf16 | al2_src f32 pair | al2_dst f32 pair | pad]
            nc.vector.tensor_copy(tb2self[:, c, 0:NCLS], ps_o[:, 0:NCLS])
            nc.scalar.copy(tb2self[:, c, NCLS: NCLS + 4].bitcast(F32),
                           ps_o[:, NCLS: NCLS + 2])
            nc.sync.dma_start(t["tb2_own"][128 * c: 128 * (c + 1), :],
                              tb2self[:, c, :])


def _l2_phase(tc, cfg, t, tb2self):
    nc = tc.nc
    nch, NCLS = cfg["nch"], cfg["NCLS"]
    K2s, toff2, TOT2 = cfg["K2s"], cfg["toff2"], cfg["TOT2"]
    nq = cfg.get("swdge_queues", 1)

    with (
        tc.tile_pool(name="gt2", bufs=cfg.get("g2_bufs", 3)) as gpool,
        tc.tile_pool(name="meta2", bufs=1) as mpool,
        tc.tile_pool(name="small2", bufs=3) as smpool,
        tc.tile_pool(name="out2p", bufs=2) as opool,
    ):
        idx_all = mpool.tile([128, 8 * TOT2], I16, tag="idx2all")
        nc.sync.dma_start(idx_all[:], t["g2idx"][:, :])
        me_all = mpool.tile([128, TOT2], F32, tag="meall")
        nc.sync.dma_start(me_all[:], t["me"][:, :])
        mo_all = mpool.tile([128, TOT2], F32, tag="moall")
        nc.sync.dma_start(mo_all[:], t["mo"][:, :])
        tb2p = t["tb2_full"].rearrange("(a b) c -> a (b c)", b=2)
        for c in range(nch):
            K = K2s[c]
            off = toff2[c]
            me = me_all[:, off: off + K]
            mo = mo_all[:, off: off + K]
            gt = gpool.tile([128, K, 128], BF16, tag="gt2")
            for b0 in range(0, K, GMAX):
                b1 = min(b0 + GMAX, K)
                nk = b1 - b0
                nc.gpsimd.dma_gather(
                    gt[:, b0:b1, :], tb2p[:, :],
                    idx_all[:, 8 * (off + b0): 8 * (off + b1)],
                    nk * 128, nk * 128, 128,
                    queue_num=_QCTR[0] % nq,
                )
                _QCTR[0] += 1
            # logits: select even/odd al2_src, add own al2_dst
            ald = tb2self[:, c, NCLS + 2: NCLS + 4].bitcast(F32)  # [128,1]
            v1 = smpool.tile([128, K], F32, tag="v1")
            nc.vector.tensor_tensor(
                v1[:], gt[:, :, NCLS: NCLS + 2].bitcast(F32)[:, :, 0], me,
                op=OP.mult)
            v2 = smpool.tile([128, K], F32, tag="v2")
            nc.vector.tensor_tensor(
                v2[:], gt[:, :, 64 + NCLS: 64 + NCLS + 2].bitcast(F32)[:, :, 0],
                mo, op=OP.mult)
            s2 = smpool.tile([128, K], F32, tag="s2")
            nc.vector.tensor_tensor(s2[:], v1[:], v2[:], op=OP.add)
            s2b = smpool.tile([128, K], F32, tag="s2b")
            nc.vector.tensor_tensor(
                s2b[:], s2[:], ald.to_broadcast([128, K]), op=OP.add)
            l2t = smpool.tile([128, K], F32, tag="l2")
            nc.vector.scalar_tensor_tensor(
                l2t[:], s2b[:], 0.2, s2b[:], op0=OP.mult, op1=OP.max)
            p2 = smpool.tile([128, K], F32, tag="p2")
            nc.scalar.activation(p2[:], l2t[:], AF.Exp)
            aE = smpool.tile([128, K], BF16, tag="aE")
            nc.vector.tensor_tensor(aE[:], p2[:], me, op=OP.mult)
            aO = smpool.tile([128, K], BF16, tag="aO")
            nc.vector.tensor_tensor(aO[:], p2[:], mo, op=OP.mult)
            # z from the already-masked weights (1-src reduces, no port lock)
            zE = smpool.tile([128, 1], F32, tag="zE")
            nc.vector.tensor_reduce(zE[:], aE[:], axis=AX.X, op=OP.add)
            zO = smpool.tile([128, 1], F32, tag="zO")
            nc.vector.tensor_reduce(zO[:], aO[:], axis=AX.X, op=OP.add)
            z2 = smpool.tile([128, 1], F32, tag="z2")
            nc.vector.tensor_tensor(z2[:], zE[:], zO[:], op=OP.add)
            # self-loop: logit from own al2_src + own al2_dst
            ss = smpool.tile([128, 1], F32, tag="ss")
            nc.vector.tensor_tensor(
                ss[:], tb2self[:, c, NCLS: NCLS + 2].bitcast(F32), ald,
                op=OP.add)
            lss = smpool.tile([128, 1], F32, tag="lss")
            nc.vector.scalar_tensor_tensor(
                lss[:], ss[:], 0.2, ss[:], op0=OP.mult, op1=OP.max)
            p2s = smpool.tile([128, 1], F32, tag="p2s")
            nc.scalar.activation(p2s[:], lss[:], AF.Exp)
            z2b = smpool.tile([128, 1], F32, tag="z2b")
            nc.vector.tensor_tensor(z2b[:], z2[:], p2s[:], op=OP.add)
            zc = smpool.tile([128, 1], F32, tag="zc")
            nc.vector.tensor_scalar_max(zc[:], z2b[:], 1e-30)
            rz2 = smpool.tile([128, 1], F32, tag="rz2")
            nc.vector.reciprocal(rz2[:], zc[:])
            # weighted messages in place, then reduce over slots
            nc.vector.tensor_tensor(
                gt[:, :, 0:NCLS], gt[:, :, 0:NCLS],
                aE[:, :, None].to_broadcast([128, K, NCLS]), op=OP.mult)
            nc.vector.tensor_tensor(
                gt[:, :, 64: 64 + NCLS], gt[:, :, 64: 64 + NCLS],
                aO[:, :, None].to_broadcast([128, K, NCLS]), op=OP.mult)
            u2a = smpool.tile([128, NCLS], F32, tag="u2a")
            nc.vector.tensor_reduce(
                u2a[:], gt[:, :, 0:NCLS].rearrange("p k f -> p f k"),
                axis=AX.X, op=OP.add)
            u2b = smpool.tile([128, NCLS], F32, tag="u2b")
            nc.vector.tensor_reduce(
                u2b[:], gt[:, :, 64: 64 + NCLS].rearrange("p k f -> p f k"),
                axis=AX.X, op=OP.add)
            u2 = smpool.tile([128, NCLS], F32, tag="u2")
            nc.vector.tensor_tensor(u2[:], u2a[:], u2b[:], op=OP.add)
            msel = smpool.tile([128, NCLS], F32, tag="msel")
            nc.vector.tensor_tensor(
                msel[:], tb2self[:, c, 0:NCLS],
                p2s[:].to_broadcast([128, NCLS]), op=OP.mult)
            u2c = smpool.tile([128, NCLS], F32, tag="u2c")
            nc.vector.tensor_tensor(u2c[:], u2[:], msel[:], op=OP.add)
            o2 = opool.tile([128, NCLS], F32, tag="o2")
            nc.vector.tensor_tensor(
                o2[:], u2c[:], rz2[:].to_broadcast([128, NCLS]), op=OP.mult)
            nc.sync.dma_start(t["out2"][128 * c: 128 * (c + 1), :], o2[:])


# ----------------------------------------------------------------------------
# PJRT execution (with on-device iteration chaining for timing)
# ----------------------------------------------------------------------------

def _pjrt_exec(nc, in_maps, n_cores, iters=1, reps=3):
    import jax
    import numpy as _np
    from jax.sharding import Mesh, PartitionSpec
    from jax.experimental.shard_map import shard_map
    from concourse import bass2jax as b2j
    from concourse import mybir as _mb

    b2j.install_neuronx_cc_hook()
    partition_name = (nc.partition_id_tensor.name
                      if nc.partition_id_tensor else None)
    in_names, out_names, out_avals, zero_outs = [], [], [], []
    for alloc in nc.m.functions[0].allocations:
        if not isinstance(alloc, _mb.MemoryLocationSet):
            continue
        name = alloc.memorylocations[0].name
        if alloc.kind == "ExternalInput":
            if name != partition_name:
                in_names.append(name)
        elif alloc.kind == "ExternalOutput":
            shape = tuple(alloc.tensor_shape)
            dtype = _mb.dt.np(alloc.dtype)
            out_names.append(name)
            out_avals.append(jax.core.ShapedArray(shape, dtype))
            zero_outs.append(_np.zeros(shape, dtype))
    n_params = len(in_names)
    all_in_names = in_names + out_names
    if partition_name is not None:
        all_in_names = all_in_names + [partition_name]

    def _body(*args):
        ins = list(args[:n_params])
        zo = list(args[n_params:])
        for _ in range(iters):
            operands = ins + zo
            if partition_name is not None:
                operands.append(b2j.partition_id_tensor())
            outs = _bass_exec_bind(b2j, operands, out_avals, all_in_names,
                                   out_names, nc)
            zo = list(outs)
        return tuple(zo)

    devices = jax.devices()[:n_cores]
    mesh = Mesh(_np.asarray(devices), ("core",))
    in_specs = (PartitionSpec("core"),) * (n_params + len(out_names))
    out_specs = (PartitionSpec("core"),) * len(out_names)
    sharded = jax.jit(shard_map(_body, mesh=mesh, in_specs=in_specs,
                                out_specs=out_specs, check_rep=False),
                      keep_unused=True)
    concat_in = [
        _np.concatenate([_np.asarray(in_maps[c][nm]) for c in range(n_cores)],
                        axis=0)
        for nm in in_names
    ]
    concat_zeros = [_np.zeros((n_cores * z.shape[0], *z.shape[1:]), z.dtype)
                    for z in zero_outs]
    import time as _time
    from jax.sharding import NamedSharding
    sh = NamedSharding(mesh, PartitionSpec("core"))
    dev_in = [jax.device_put(a, sh) for a in concat_in]
    dev_zeros = [jax.device_put(a, sh) for a in concat_zeros]
    jax.block_until_ready(dev_in + dev_zeros)
    out_arrs = sharded(*dev_in, *dev_zeros)
    jax.block_until_ready(out_arrs)
    times = []
    for _ in range(reps):
        t0 = _time.perf_counter()
        out_arrs = sharded(*dev_in, *dev_zeros)
        jax.block_until_ready(out_arrs)
        times.append(_time.perf_counter() - t0)
    dt = min(times)
    results = [
        {nm: _np.asarray(out_arrs[i]).reshape(n_cores, *out_avals[i].shape)[c]
         for i, nm in enumerate(out_names)}
        for c in range(n_cores)
    ]
    return results, dt


def _bass_exec_bind(b2j, operands, out_avals, in_names, out_names, nc):
    return b2j._bass_exec_p.bind(
        *operands,
        out_avals=tuple(out_avals),
        in_names=tuple(in_names),
        out_names=tuple(out_names),
        lowering_input_output_aliases=(),
        sim_require_finite=True,
        sim_require_nnan=True,
        nc=nc,
    )


# ----------------------------------------------------------------------------
# Entry point
# ----------------------------------------------------------------------------

_CACHE = {}


def _run(inputs, trace=False):
    x = np.asarray(inputs["x"], np.float32)
    edge_index = np.asarray(inputs["edge_index"], np.int32)
    W1 = np.asarray(inputs["W1"], np.float32)
    a1s = np.asarray(inputs["att1_src"], np.float32)
    a1d = np.asarray(inputs["att1_dst"], np.float32)
    W2 = np.asarray(inputs["W2"], np.float32)
    a2s = np.asarray(inputs["att2_src"], np.float32)
    a2d = np.asarray(inputs["att2_dst"], np.float32)
    b1 = np.asarray(inputs["b1"], np.float32)
    b2 = np.asarray(inputs["b2"], np.float32)
    assert not b1.any() and not b2.any(), "nonzero bias unsupported"

    key = hashlib.sha1(
        b"v3" + edge_index.tobytes() + np.int64(x.shape).tobytes()
    ).hexdigest()
    cfg, shared, per_core = _host_prep(x, edge_index, W1, a1s, a1d, W2, a2s, a2d)
    if key not in _CACHE:
        _CACHE[key] = _build_program(cfg)
    nc = _CACHE[key]

    in_maps = []
    for k in range(cfg["n_cores"]):
        m = dict(shared)
        m.update(per_core[k])
        in_maps.append(m)
    res = run_bass_kernel_spmd(nc, in_maps, list(range(cfg["n_cores"])),
                               trace=trace)
    out = gather_out([res.results[k]["out2"] for k in range(cfg["n_cores"])],
                     cfg)
    return out.astype(np.float32), res


def gather_out(outs, cfg):
    allrows = np.concatenate(outs, axis=0)          # [R, NCLS] permuted
    return allrows[cfg["pos"][: cfg["N"]]]


def kernel(**inputs):
    out, _ = _run(inputs, trace=False)
    return out


# revision 34
# speedup vs baseline: 2.5222x; 1.0714x over previous
"""GAT (2-layer, PyG-default) Trainium2 Bass kernel, 8-core SPMD.

v3 — trace-driven rework of the dst-major design (baseline 2.0ms ->
~1.56ms).  Measured constraints that shaped it: every gathered row is
one SWDGE descriptor costing ~70-90ns of SDMA-engine time regardless
of size (the kernel is descriptor-count-bound, ~230k descs/core), a
dma_gather call with >1024 indices wedges the device, calls with >512
descriptors block the GpSimd engine until ring space frees, and any
2-input DVE op holds the SBUF port pair that SWDGE descriptor
generation needs (DVE TENSOR_TENSOR time stalls the gather pipe).

  - Node permutation is globally degree-balanced: nodes ranked by
    in-degree are dealt into (chunk, core, lane) so each chunk's 1024
    lanes (128 per core x 8 cores) have near-uniform degree, shrinking
    the per-chunk slot count K toward the mean degree.
  - L1 rows are stored feature-transposed (c,h): the per-edge softmax
    weight broadcast then has unit inner stride on every operand, so
    the big per-round message multiply runs in DVE 2x_1P mode
    (in-place on the gather tile; no per-slot DVE ops).
  - L1 chunks are processed in SLAB-slot rounds: small gather tiles
    give a deep (6-buffer) gather pipeline, gather calls stay at <=4
    slots (512 descriptors, fire-and-forget), and each DVE-lock piece
    is ~3us so the 4 SWDGE queue rings (~9us of buffered descriptors)
    ride through it.  PSUM accumulates across rounds.
  - L2 gathers PAIRS of compact 128B tb2 rows (idx = src>>1, 256B
    descriptors): the pair index range (25088 < 32768) fits one int16
    window, killing L1's 4-window slot inflation for L2.  Even/odd row
    selection is folded into the alpha masks (me/mo).  L2 self-loop
    contributions come from an SBUF-resident tb2self captured while L1
    writes tb2 rows, so they never touch the gather path.
  - Phase 0 splits the PSUM->bf16 casts 3:1 between Vector and Scalar;
    transposes and the W2 projection run in bf16.

Self-contained: only needs numpy + the concourse tree at /opt/trn_rl_repo.
"""

import hashlib
import sys

import numpy as np

for _p in ("/opt/trn_rl_repo",):
    if _p not in sys.path:
        sys.path.insert(0, _p)

import concourse.bacc as bacc
import concourse.bass as bass
import concourse.tile as tile
from concourse import mybir
from concourse.bass_utils import run_bass_kernel_spmd

F32 = mybir.dt.float32
BF16 = mybir.dt.bfloat16
I16 = mybir.dt.int16
AF = mybir.ActivationFunctionType
OP = mybir.AluOpType
AX = mybir.AxisListType

N_CORES = 8
SPAN = 32768
W = 4
GMAX = 4
SLAB = 12  # L1 slots per gather tile / DVE-multiply piece (multiple of GMAX)
_QCTR = [0]  # global SWDGE queue round-robin


# ----------------------------------------------------------------------------
# Host-side edge planning
# ----------------------------------------------------------------------------

def _wrap_idx(si, n_cores, K):
    """[n_cores,128,K] int -> [n_cores,16,8K] in the dma_gather idx layout
    (idx of token T, partition p lands at [p%16, 8*T + p//16])."""
    tmp = si.reshape(n_cores, 8, 16, K)
    return np.ascontiguousarray(tmp.transpose(0, 2, 3, 1)).reshape(
        n_cores, 16, 8 * K)


def _edge_plan(edge_index, N, n_cores, nch, npcp):
    """Degree-balanced dst-major plans for both layers.

    L1: self-loops included, W=4 overlapping 32768-row windows.
    L2: self-loops excluded, rows gathered in PAIRS (idx = srow>>1) from
        a single window, with even/odd masks me/mo.
    """
    R = n_cores * npcp
    bases = [round(q * (R - SPAN) / (W - 1)) for q in range(W)]

    src1 = np.concatenate([np.asarray(edge_index[0], np.int64), np.arange(N)])
    dst1 = np.concatenate([np.asarray(edge_index[1], np.int64), np.arange(N)])
    deg = np.bincount(dst1, minlength=R)
    # Deal degree-ranked nodes into (chunk, core, lane): chunk lanes get
    # near-uniform degree across all cores.
    order = np.argsort(deg, kind="stable")
    blk = 128 * n_cores
    ii = np.arange(R)
    chunk = ii // blk
    core = (ii % blk) // 128
    lane = ii % 128
    pos = np.empty(R, np.int64)
    pos[order] = core * npcp + chunk * 128 + lane

    def build_emat(src, dst):
        srow = pos[src]
        dpos = pos[dst]
        key = dpos // npcp * (nch * 128) + dpos % npcp
        order_e = np.lexsort((srow, key))
        ks, ss = key[order_e], srow[order_e]
        counts = np.bincount(ks, minlength=n_cores * nch * 128)
        maxd = max(int(counts.max()), 1)
        starts = np.zeros(len(counts) + 1, np.int64)
        np.cumsum(counts, out=starts[1:])
        col = np.arange(len(ss)) - starts[ks]
        E = np.full((n_cores * nch * 128, maxd), 2**31, np.int64)
        E[ks, col] = ss
        return E, counts

    Emat1, counts1 = build_emat(src1, dst1)
    Emat2, counts2 = build_emat(np.asarray(edge_index[0], np.int64),
                                np.asarray(edge_index[1], np.int64))

    def plan_chunk(E, degv):
        valid = E < 2**31
        A = np.zeros(W, np.int64)
        B = np.zeros(W, np.int64)
        dmax = int(degv.max())
        for q in range(W - 1):
            A[q] = int(((E < bases[q + 1]) & valid).sum(axis=1).max())
            B[q] = int(((E >= bases[q] + SPAN) & valid).sum(axis=1).max())
        A[W - 1] = dmax
        K = int(max(dmax, (A + B).max(), 1))
        L = E.shape[0]
        maxd = E.shape[1]
        while True:
            P = np.maximum.accumulate(np.minimum(np.maximum(A, 0), K - B))
            P[W - 1] = K
            n = np.diff(np.concatenate([[0], P]))
            qcls = np.repeat(np.arange(W), n)
            ptr = np.zeros(L, np.int64)
            slotidx = np.zeros((L, K), np.int32)
            slotmask = np.zeros((L, K), bool)
            ok = True
            for t in range(K):
                b = bases[qcls[t]]
                cur = E[np.arange(L), np.minimum(ptr, maxd - 1)]
                vv = ptr < degv
                if np.any(vv & (cur < b)):
                    ok = False
                    break
                fit = vv & (cur >= b) & (cur < b + SPAN)
                slotidx[:, t] = np.where(fit, cur - b, 0)
                slotmask[:, t] = fit
                ptr += fit
            if ok and np.all(ptr == degv):
                return K, qcls, slotidx, slotmask
            K += 1
            assert K < dmax + 24, "edge window planning failed to converge"

    Ks, toff, calls = [], [], []
    blocks_idx, blocks_mask = [], []
    K2s, toff2 = [], []
    blocks_idx2, blocks_me, blocks_mo = [], [], []
    off = 0
    off2 = 0
    for c in range(nch):
        lanes = ((np.arange(n_cores)[:, None] * nch + c) * 128
                 + np.arange(128)[None, :]).ravel()
        # ---- L1 (windowed, self-loops in-plan) ----
        K, qcls, si, sm = plan_chunk(Emat1[lanes], counts1[lanes])
        Ks.append(K)
        toff.append(off)
        cc = []
        t0 = 0
        while t0 < K:
            q = qcls[t0]
            t1 = t0
            nxt_slab = (t0 // SLAB + 1) * SLAB
            while (t1 < K and qcls[t1] == q and t1 - t0 < GMAX
                   and t1 < nxt_slab):
                t1 += 1
            cc.append((t0, t1, int(q)))
            t0 = t1
        calls.append(cc)
        blocks_idx.append(_wrap_idx(si.reshape(n_cores, 128, K), n_cores, K))
        blocks_mask.append(sm.reshape(n_cores, 128, K))
        off += K
        # ---- L2 (paired rows, single window, no self-loops) ----
        E2 = Emat2[lanes]
        degv2 = counts2[lanes]
        K2 = max(int(degv2.max()), 1)
        sub = E2[:, :K2]
        valid = np.arange(K2)[None, :] < degv2[:, None]
        idx2 = np.where(valid, sub >> 1, 0).astype(np.int32)
        par = np.where(valid, sub & 1, 0)
        me = (valid & (par == 0)).astype(np.float32)
        mo = (valid & (par == 1)).astype(np.float32)
        K2s.append(K2)
        toff2.append(off2)
        blocks_idx2.append(_wrap_idx(idx2.reshape(n_cores, 128, K2),
                                     n_cores, K2))
        blocks_me.append(me.reshape(n_cores, 128, K2))
        blocks_mo.append(mo.reshape(n_cores, 128, K2))
        off2 += K2
    TOT, TOT2 = off, off2
    idx16 = np.tile(np.concatenate(blocks_idx, axis=2).astype(np.int16),
                    (1, 8, 1))
    mask = np.concatenate(blocks_mask, axis=2).astype(np.float32)
    idx16_2 = np.tile(np.concatenate(blocks_idx2, axis=2).astype(np.int16),
                      (1, 8, 1))
    me_all = np.concatenate(blocks_me, axis=2)
    mo_all = np.concatenate(blocks_mo, axis=2)
    return (pos, bases, Ks, toff, TOT, calls, idx16, mask,
            K2s, toff2, TOT2, idx16_2, me_all, mo_all)


def _host_prep(x, edge_index, W1, att1_src, att1_dst, W2, att2_src, att2_dst):
    N, F = x.shape
    H, C = att1_src.shape
    HC = H * C
    NCLS = W2.shape[1]
    n_cores = N_CORES
    nch = -(-N // (n_cores * 128))
    npcp = nch * 128
    R = n_cores * npcp

    (pos, bases, Ks, toff, TOT, calls, idx16, mask,
     K2s, toff2, TOT2, idx16_2, me_all, mo_all) = _edge_plan(
        edge_index, N, n_cores, nch, npcp)

    # (c,h) feature permutation: new col c*H+h <- old col h*C+c
    jj = np.arange(HC)
    perm = (jj % H) * C + jj // H
    W1p = W1[:, perm]

    # Folded attention-logit weight columns (independent of column order)
    Wa_s = np.einsum("fhc,hc->fh", W1.reshape(F, H, C), att1_src).astype(np.float32)
    Wa_d = np.einsum("fhc,hc->fh", W1.reshape(F, H, C), att1_dst).astype(np.float32)
    W1e = np.ascontiguousarray(
        np.concatenate([W1p, Wa_s, Wa_d], axis=1), dtype=np.float32)  # [F, 528]

    w2s = (W2 @ att2_src[0]).astype(np.float32)
    w2d = (W2 @ att2_dst[0]).astype(np.float32)
    W2e_flat = np.zeros((HC, 64), np.float32)
    W2e_flat[:, :NCLS] = W2[perm]
    W2e_flat[:, NCLS] = w2s[perm]
    W2e_flat[:, NCLS + 1] = w2d[perm]
    nslab = HC // 128
    W2e = np.ascontiguousarray(
        W2e_flat.reshape(nslab, 128, 64).transpose(1, 0, 2))  # [128, 4, 64]

    import ml_dtypes
    bf = ml_dtypes.bfloat16
    xtab = np.zeros((R, F), np.float32)
    xtab[pos[np.arange(N)]] = x
    xTp = np.ascontiguousarray(xtab.T).astype(bf)   # [F, R] permuted cols
    W1e = W1e.astype(bf)
    ident = np.eye(128, dtype=np.float32).astype(bf)

    cfg = dict(
        N=N, F=F, H=H, C=C, HC=HC, NCLS=NCLS, n_cores=n_cores,
        nch=nch, npcp=npcp, R=R, nslab=nslab,
        Ks=Ks, toff=toff, TOT=TOT, calls=calls, bases=bases, pos=pos,
        K2s=K2s, toff2=toff2, TOT2=TOT2,
        swdge_queues=4, p0_bufs=4,
        gt_bufs=6, g2_bufs=4,
    )
    shared = dict(xTp=xTp, W1e=W1e, W2e=W2e.astype(bf), ident=ident)
    per_core = [
        dict(g1idx=idx16[k], mask=mask[k].astype(bf),
             g2idx=idx16_2[k], me=me_all[k], mo=mo_all[k])
        for k in range(n_cores)
    ]
    return cfg, shared, per_core


# ----------------------------------------------------------------------------
# Device program
# ----------------------------------------------------------------------------

def _build_program(cfg):
    F, HC, NCLS = cfg["F"], cfg["HC"], cfg["NCLS"]
    n_cores, npcp, R = cfg["n_cores"], cfg["npcp"], cfg["R"]
    nslab, TOT, TOT2 = cfg["nslab"], cfg["TOT"], cfg["TOT2"]

    nc = bacc.Bacc("TRN2", target_bir_lowering=False, debug=False,
                   num_devices=n_cores,
                   num_swdge_queues=cfg.get("swdge_queues", 1))

    xTp = nc.dram_tensor("xTp", [F, R], BF16, kind="ExternalInput").ap()
    W1e = nc.dram_tensor("W1e", [F, HC + 16], BF16, kind="ExternalInput").ap()
    W2e = nc.dram_tensor("W2e", [128, nslab, 64], BF16, kind="ExternalInput").ap()
    ident_d = nc.dram_tensor("ident", [128, 128], BF16, kind="ExternalInput").ap()
    g1idx = nc.dram_tensor("g1idx", [128, 8 * TOT], I16,
                           kind="ExternalInput").ap()
    mask_d = nc.dram_tensor("mask", [128, TOT], BF16, kind="ExternalInput").ap()
    g2idx = nc.dram_tensor("g2idx", [128, 8 * TOT2], I16,
                           kind="ExternalInput").ap()
    me_d = nc.dram_tensor("me", [128, TOT2], F32, kind="ExternalInput").ap()
    mo_d = nc.dram_tensor("mo", [128, TOT2], F32, kind="ExternalInput").ap()

    T1 = nc.dram_tensor("T1", [R, 640], BF16).ap()
    tb2_own = nc.dram_tensor("tb2_own", [npcp, 64], BF16).ap()
    tb2_full = nc.dram_tensor("tb2_full", [R, 64], BF16,
                              addr_space="Shared").ap()
    out2 = nc.dram_tensor("out2", [npcp, NCLS], F32, kind="ExternalOutput").ap()

    tensors = dict(xTp=xTp, W1e=W1e, W2e=W2e, ident=ident_d, g1idx=g1idx,
                   mask=mask_d, g2idx=g2idx, me=me_d, mo=mo_d,
                   T1=T1, tb2_own=tb2_own, tb2_full=tb2_full, out2=out2)
    repeat = cfg.get("repeat", 1)
    with tile.TileContext(nc) as tc:
        for _ in range(repeat):
            _emit(tc, cfg, tensors)
    nc.compile()
    return nc


def _emit(tc, cfg, t):
    nc = tc.nc
    H, HC, NCLS = cfg["H"], cfg["HC"], cfg["NCLS"]
    n_cores, nch, npcp, R = cfg["n_cores"], cfg["nch"], cfg["npcp"], cfg["R"]
    nslab = cfg["nslab"]
    NTB = R // 128

    with tc.tile_pool(name="consts", bufs=1) as cpool:
        W1e_sb = cpool.tile([128, HC + 16], BF16)
        nc.sync.dma_start(W1e_sb[:], t["W1e"][:, :])
        W2e_sb = cpool.tile([128, nslab, 64], BF16)
        nc.sync.dma_start(W2e_sb[:], t["W2e"][:, :, :])
        ident_bf = cpool.tile([128, 128], BF16)
        nc.sync.dma_start(ident_bf[:], t["ident"][:, :])
        ald1_all = cpool.tile([128, NTB, H], F32)
        ald1_sb = cpool.tile([128, nch, H], F32)
        tb2self = cpool.tile([128, nch, 64], BF16)

        # ---------------- Phase 0: permuted node table T1 ----------------
        with (
            nc.named_scope("p0"),
            tc.tile_pool(name="p0", bufs=cfg.get("p0_bufs", 4)) as pool,
            tc.tile_pool(name="p0ps", bufs=cfg.get("p0_bufs", 4),
                         space="PSUM") as pps,
            tc.tile_pool(name="p0psb", bufs=2, space="PSUM") as ppsb,
        ):
            nblk = R // 512
            for i in range(nblk):
                xt = pool.tile([128, 512], BF16, tag="xt")
                nc.sync.dma_start(xt[:], t["xTp"][:, 512 * i: 512 * i + 512])
                rowB = pool.tile([128, 4, HC + 16], BF16, tag="rowB")
                # all 4 j-groups' attention-logit cols land in one PSUM tile
                # so the small copies batch to 2 per block instead of 8
                psB = ppsb.tile([128, 4, 16], F32, tag="psB")
                for j in range(4):
                    ps = pps.tile([128, HC], F32, tag="ps")
                    nc.tensor.matmul(ps[:],
                                     lhsT=xt[:, 128 * j: 128 * j + 128],
                                     rhs=W1e_sb[:, 0:HC], start=True, stop=True)
                    nc.tensor.matmul(psB[:, j, :],
                                     lhsT=xt[:, 128 * j: 128 * j + 128],
                                     rhs=W1e_sb[:, HC: HC + 16],
                                     start=True, stop=True)
                    if j < 3:
                        nc.vector.tensor_copy(rowB[:, j, 0:HC], ps[:, 0:HC])
                    else:
                        nc.scalar.copy(rowB[:, j, 0:HC], ps[:, 0:HC])
                nc.vector.tensor_copy(
                    rowB[:, :, HC: HC + 16].bitcast(F32),
                    psB[:, :, 0:H])
                nc.scalar.copy(ald1_all[:, 4 * i: 4 * i + 4, :],
                               psB[:, :, H: 2 * H])
                # ACT HWDGE ring: xt prefetches must not queue behind the
                # rowB-ready waits of T1 writes on the SP ring
                nc.scalar.dma_start(
                    t["T1"][512 * i: 512 * i + 512, 0: HC + 16].rearrange(
                        "(j p) c -> p j c", p=128),
                    rowB[:],
                )

        pid = nc.partition_id()
        nc.sync.dma_start(ald1_sb[:], ald1_all[:, bass.ds(pid * nch, nch), :])

        if cfg.get("phases", "full") == "p0":
            return
        # ---------------- L1 edge phase ----------------
        with nc.named_scope("l1"):
            _l1_phase(tc, cfg, t, W2e_sb, ident_bf, ald1_sb, tb2self)

        if cfg.get("phases", "full") == "p0+l1":
            return
        # ---------------- allgather ----------------
        with nc.named_scope("ag"):
            if cfg.get("no_collective"):
                nc.sync.dma_start(t["tb2_full"][0:npcp, :], t["tb2_own"][:, :])
            else:
                nc.gpsimd.collective_compute(
                    "AllGather",
                    OP.bypass,
                    replica_groups=[list(range(n_cores))],
                    ins=[t["tb2_own"][:, :]],
                    outs=[t["tb2_full"][:, :]],
                )

        if cfg.get("phases", "full") == "p0+l1+ag":
            return
        # ---------------- L2 edge phase ----------------
        with nc.named_scope("l2"):
            _l2_phase(tc, cfg, t, tb2self)


def _l1_phase(tc, cfg, t, W2e_sb, ident_bf, ald1_sb, tb2self):
    nc = tc.nc
    nch, H, C, HC, NCLS = cfg["nch"], cfg["H"], cfg["C"], cfg["HC"], cfg["NCLS"]
    nslab = cfg["nslab"]
    Ks, toff, calls, bases = cfg["Ks"], cfg["toff"], cfg["calls"], cfg["bases"]
    TOT = cfg["TOT"]
    nq = cfg.get("swdge_queues", 1)

    with (
        tc.tile_pool(name="gt1", bufs=cfg.get("gt_bufs", 2)) as gpool,
        tc.tile_pool(name="meta1", bufs=1) as mpool,
        tc.tile_pool(name="small1", bufs=3) as smpool,
        tc.tile_pool(name="out1", bufs=2) as opool,
        tc.tile_pool(name="ps_u1", bufs=2, space="PSUM") as pp_u,
        tc.tile_pool(name="ps_tr1", bufs=2, space="PSUM") as pp_tr,
        tc.tile_pool(name="ps_o1", bufs=2, space="PSUM") as pp_o,
        tc.tile_pool(name="ps_a1", bufs=2, space="PSUM") as pp_a,
    ):
        idx_all = mpool.tile([128, 8 * TOT], I16, tag="idxall")
        nc.sync.dma_start(idx_all[:], t["g1idx"][:, :])
        msk_all = mpool.tile([128, TOT], BF16, tag="mskall")
        nc.sync.dma_start(msk_all[:], t["mask"][:, :])
        for c in range(nch):
            K = Ks[c]
            off = toff[c]
            stop = cfg.get("l1_stop")
            p_bf = smpool.tile([128, K, H], BF16, tag="pbf")
            ps_u = pp_u.tile([128, HC], F32, tag="u")
            # slab rounds: small gather tiles (deep pipelining) and short
            # DVE-lock pieces so SWDGE descriptor generation stays fed
            for r0 in range(0, K, SLAB):
                r1 = min(r0 + SLAB, K)
                kr = r1 - r0
                gt = gpool.tile([128, SLAB, 640], BF16, tag="gt")
                for (b0, b1, q) in calls[c]:
                    if b0 < r0 or b0 >= r1:
                        continue
                    nk = b1 - b0
                    nc.gpsimd.dma_gather(
                        gt[:, b0 - r0: b1 - r0, :],
                        t["T1"][bases[q]: bases[q] + SPAN, :],
                        idx_all[:, 8 * (off + b0): 8 * (off + b1)],
                        nk * 128, nk * 128, 640,
                        queue_num=_QCTR[0] % nq,
                    )
                    _QCTR[0] += 1
                if stop == "gather":
                    continue
                # p = exp(leakyrelu(al_src[src] + al_dst[dst])) * mask
                s_t = smpool.tile([128, SLAB, H], F32, tag="s")
                nc.vector.tensor_tensor(
                    s_t[:, 0:kr, :],
                    gt[:, 0:kr, HC: HC + 16].bitcast(F32),
                    ald1_sb[:, c, None, :].to_broadcast([128, kr, H]),
                    op=OP.add,
                )
                l_t = smpool.tile([128, SLAB, H], F32, tag="l")
                nc.vector.scalar_tensor_tensor(
                    l_t[:, 0:kr, :], s_t[:, 0:kr, :], 0.2, s_t[:, 0:kr, :],
                    op0=OP.mult, op1=OP.max
                )
                p_t = smpool.tile([128, SLAB, H], F32, tag="p")
                nc.scalar.activation(p_t[:, 0:kr, :], l_t[:, 0:kr, :], AF.Exp)
                nc.vector.tensor_tensor(
                    p_bf[:, r0:r1, :], p_t[:, 0:kr, :],
                    msk_all[:, off + r0: off + r1, None].to_broadcast(
                        [128, kr, H]),
                    op=OP.mult,
                )
                if stop in ("logits", "mult", "agg"):
                    continue
                # in-place alpha multiply, (c,h) layout -> unit inner stride.
                # alpha staged into PSUM via ACT: the TT then holds only one
                # SBUF read port, so SWDGE descriptor-gen is not locked out.
                if cfg.get("alpha_psum", False):
                    pa = pp_a.tile([128, SLAB, H], BF16, tag="pa")
                    nc.scalar.copy(pa[:, 0:kr, :], p_bf[:, r0:r1, :])
                    alpha_src = pa[:, 0:kr, None, :]
                else:
                    alpha_src = p_bf[:, r0:r1, None, :]
                nc.vector.tensor_tensor(
                    gt[:, 0:kr, 0:HC].rearrange("p k (c h) -> p k c h", h=H),
                    gt[:, 0:kr, 0:HC].rearrange("p k (c h) -> p k c h", h=H),
                    alpha_src.to_broadcast([128, kr, C, H]),
                    op=OP.mult,
                )
                for k in range(kr):
                    nc.tensor.matmul(
                        ps_u[:], lhsT=ident_bf[:], rhs=gt[:, k, 0:HC],
                        start=(r0 + k == 0), stop=(r0 + k == K - 1),
                    )
            if stop in ("gather", "logits", "mult", "agg"):
                continue
            zr = smpool.tile([128, H], F32, tag="zr")
            nc.vector.tensor_reduce(
                zr[:], p_bf[:].rearrange("p k h -> p h k"),
                axis=AX.X, op=OP.add,
            )
            zb = smpool.tile([128, H], F32, tag="zb")
            nc.vector.tensor_scalar_max(zb[:], zr[:], 1e-30)
            rz = smpool.tile([128, H], F32, tag="rz")
            nc.vector.reciprocal(rz[:], zb[:])
            h2 = opool.tile([128, HC], F32, tag="h2")
            nc.vector.tensor_tensor(
                h2[:].rearrange("p (c h) -> p c h", h=H),
                ps_u[:].rearrange("p (c h) -> p c h", h=H),
                rz[:, None, :].to_broadcast([128, C, H]),
                op=OP.mult,
            )
            h2r = opool.tile([128, HC], BF16, tag="h2r")
            nc.scalar.activation(h2r[:], h2[:], AF.Relu)
            if stop == "h2":
                continue
            ps_o = pp_o.tile([128, 64], F32, tag="o")
            for j in range(nslab):
                ps_tr = pp_tr.tile([128, 128], BF16, tag="tr")
                nc.tensor.transpose(
                    ps_tr[:], h2r[:, 128 * j: 128 * (j + 1)], ident_bf[:]
                )
                h2t = smpool.tile([128, 128], BF16, tag="h2t")
                nc.scalar.copy(h2t[:], ps_tr[:])
                nc.tensor.matmul(
                    ps_o[:], lhsT=h2t[:], rhs=W2e_sb[:, j, :],
                    start=(j == 0), stop=(j == nslab - 1),
                )
            # tb2 row: [40 cls bf16 | al2_src f32 pair | al2_dst f32 pair | pad]
            nc.vector.tensor_copy(tb2self[:, c, 0:NCLS], ps_o[:, 0:NCLS])
            nc.scalar.copy(tb2self[:, c, NCLS: NCLS + 4].bitcast(F32),
                           ps_o[:, NCLS: NCLS + 2])
            nc.sync.dma_start(t["tb2_own"][128 * c: 128 * (c + 1), :],
                              tb2self[:, c, :])


def _l2_phase(tc, cfg, t, tb2self):
    nc = tc.nc
    nch, NCLS = cfg["nch"], cfg["NCLS"]
    K2s, toff2, TOT2 = cfg["K2s"], cfg["toff2"], cfg["TOT2"]
    nq = cfg.get("swdge_queues", 1)

    with (
        tc.tile_pool(name="gt2", bufs=cfg.get("g2_bufs", 3)) as gpool,
        tc.tile_pool(name="meta2", bufs=1) as mpool,
        tc.tile_pool(name="small2", bufs=3) as smpool,
        tc.tile_pool(name="out2p", bufs=2) as opool,
    ):
        idx_all = mpool.tile([128, 8 * TOT2], I16, tag="idx2all")
        nc.sync.dma_start(idx_all[:], t["g2idx"][:, :])
        me_all = mpool.tile([128, TOT2], F32, tag="meall")
        nc.sync.dma_start(me_all[:], t["me"][:, :])
        mo_all = mpool.tile([128, TOT2], F32, tag="moall")
        nc.sync.dma_start(mo_all[:], t["mo"][:, :])
        tb2p = t["tb2_full"].rearrange("(a b) c -> a (b c)", b=2)
        for c in range(nch):
            K = K2s[c]
            off = toff2[c]
            me = me_all[:, off: off + K]
            mo = mo_all[:, off: off + K]
            gt = gpool.tile([128, K, 128], BF16, tag="gt2")
            for b0 in range(0, K, GMAX):
                b1 = min(b0 + GMAX, K)
                nk = b1 - b0
                nc.gpsimd.dma_gather(
                    gt[:, b0:b1, :], tb2p[:, :],
                    idx_all[:, 8 * (off + b0): 8 * (off + b1)],
                    nk * 128, nk * 128, 128,
                    queue_num=_QCTR[0] % nq,
                )
                _QCTR[0] += 1
            # logits: select even/odd al2_src, add own al2_dst
            ald = tb2self[:, c, NCLS + 2: NCLS + 4].bitcast(F32)  # [128,1]
            v1 = smpool.tile([128, K], F32, tag="v1")
            nc.vector.tensor_tensor(
                v1[:], gt[:, :, NCLS: NCLS + 2].bitcast(F32)[:, :, 0], me,
                op=OP.mult)
            v2 = smpool.tile([128, K], F32, tag="v2")
            nc.vector.tensor_tensor(
                v2[:], gt[:, :, 64 + NCLS: 64 + NCLS + 2].bitcast(F32)[:, :, 0],
                mo, op=OP.mult)
            s2 = smpool.tile([128, K], F32, tag="s2")
            nc.vector.tensor_tensor(s2[:], v1[:], v2[:], op=OP.add)
            s2b = smpool.tile([128, K], F32, tag="s2b")
            nc.vector.tensor_tensor(
                s2b[:], s2[:], ald.to_broadcast([128, K]), op=OP.add)
            l2t = smpool.tile([128, K], F32, tag="l2")
            nc.vector.scalar_tensor_tensor(
                l2t[:], s2b[:], 0.2, s2b[:], op0=OP.mult, op1=OP.max)
            p2 = smpool.tile([128, K], F32, tag="p2")
            nc.scalar.activation(p2[:], l2t[:], AF.Exp)
            aE = smpool.tile([128, K], BF16, tag="aE")
            nc.vector.tensor_tensor(aE[:], p2[:], me, op=OP.mult)
            aO = smpool.tile([128, K], BF16, tag="aO")
            nc.vector.tensor_tensor(aO[:], p2[:], mo, op=OP.mult)
            # z from the already-masked weights (1-src reduces, no port lock)
            zE = smpool.tile([128, 1], F32, tag="zE")
            nc.vector.tensor_reduce(zE[:], aE[:], axis=AX.X, op=OP.add)
            zO = smpool.tile([128, 1], F32, tag="zO")
            nc.vector.tensor_reduce(zO[:], aO[:], axis=AX.X, op=OP.add)
            z2 = smpool.tile([128, 1], F32, tag="z2")
            nc.vector.tensor_tensor(z2[:], zE[:], zO[:], op=OP.add)
            # self-loop: logit from own al2_src + own al2_dst
            ss = smpool.tile([128, 1], F32, tag="ss")
            nc.vector.tensor_tensor(
                ss[:], tb2self[:, c, NCLS: NCLS + 2].bitcast(F32), ald,
                op=OP.add)
            lss = smpool.tile([128, 1], F32, tag="lss")
            nc.vector.scalar_tensor_tensor(
                lss[:], ss[:], 0.2, ss[:], op0=OP.mult, op1=OP.max)
            p2s = smpool.tile([128, 1], F32, tag="p2s")
            nc.scalar.activation(p2s[:], lss[:], AF.Exp)
            z2b = smpool.tile([128, 1], F32, tag="z2b")
            nc.vector.tensor_tensor(z2b[:], z2[:], p2s[:], op=OP.add)
            zc = smpool.tile([128, 1], F32, tag="zc")
            nc.vector.tensor_scalar_max(zc[:], z2b[:], 1e-30)
            rz2 = smpool.tile([128, 1], F32, tag="rz2")
            nc.vector.reciprocal(rz2[:], zc[:])
            # weighted messages in place, then reduce over slots
            nc.vector.tensor_tensor(
                gt[:, :, 0:NCLS], gt[:, :, 0:NCLS],
                aE[:, :, None].to_broadcast([128, K, NCLS]), op=OP.mult)
            nc.vector.tensor_tensor(
                gt[:, :, 64: 64 + NCLS], gt[:, :, 64: 64 + NCLS],
                aO[:, :, None].to_broadcast([128, K, NCLS]), op=OP.mult)
            u2a = smpool.tile([128, NCLS], F32, tag="u2a")
            nc.vector.tensor_reduce(
                u2a[:], gt[:, :, 0:NCLS].rearrange("p k f -> p f k"),
                axis=AX.X, op=OP.add)
            u2b = smpool.tile([128, NCLS], F32, tag="u2b")
            nc.vector.tensor_reduce(
                u2b[:], gt[:, :, 64: 64 + NCLS].rearrange("p k f -> p f k"),
                axis=AX.X, op=OP.add)
            u2 = smpool.tile([128, NCLS], F32, tag="u2")
            nc.vector.tensor_tensor(u2[:], u2a[:], u2b[:], op=OP.add)
            msel = smpool.tile([128, NCLS], F32, tag="msel")
            nc.vector.tensor_tensor(
                msel[:], tb2self[:, c, 0:NCLS],
                p2s[:].to_broadcast([128, NCLS]), op=OP.mult)
            u2c = smpool.tile([128, NCLS], F32, tag="u2c")
            nc.vector.tensor_tensor(u2c[:], u2[:], msel[:], op=OP.add)
            o2 = opool.tile([128, NCLS], F32, tag="o2")
            nc.vector.tensor_tensor(
                o2[:], u2c[:], rz2[:].to_broadcast([128, NCLS]), op=OP.mult)
            nc.sync.dma_start(t["out2"][128 * c: 128 * (c + 1), :], o2[:])


# ----------------------------------------------------------------------------
# PJRT execution (with on-device iteration chaining for timing)
# ----------------------------------------------------------------------------

def _pjrt_exec(nc, in_maps, n_cores, iters=1, reps=3):
    import jax
    import numpy as _np
    from jax.sharding import Mesh, PartitionSpec
    from jax.experimental.shard_map import shard_map
    from concourse import bass2jax as b2j
    from concourse import mybir as _mb

    b2j.install_neuronx_cc_hook()
    partition_name = (nc.partition_id_tensor.name
                      if nc.partition_id_tensor else None)
    in_names, out_names, out_avals, zero_outs = [], [], [], []
    for alloc in nc.m.functions[0].allocations:
        if not isinstance(alloc, _mb.MemoryLocationSet):
            continue
        name = alloc.memorylocations[0].name
        if alloc.kind == "ExternalInput":
            if name != partition_name:
                in_names.append(name)
        elif alloc.kind == "ExternalOutput":
            shape = tuple(alloc.tensor_shape)
            dtype = _mb.dt.np(alloc.dtype)
            out_names.append(name)
            out_avals.append(jax.core.ShapedArray(shape, dtype))
            zero_outs.append(_np.zeros(shape, dtype))
    n_params = len(in_names)
    all_in_names = in_names + out_names
    if partition_name is not None:
        all_in_names = all_in_names + [partition_name]

    def _body(*args):
        ins = list(args[:n_params])
        zo = list(args[n_params:])
        for _ in range(iters):
            operands = ins + zo
            if partition_name is not None:
                operands.append(b2j.partition_id_tensor())
            outs = _bass_exec_bind(b2j, operands, out_avals, all_in_names,
                                   out_names, nc)
            zo = list(outs)
        return tuple(zo)

    devices = jax.devices()[:n_cores]
    mesh = Mesh(_np.asarray(devices), ("core",))
    in_specs = (PartitionSpec("core"),) * (n_params + len(out_names))
    out_specs = (PartitionSpec("core"),) * len(out_names)
    sharded = jax.jit(shard_map(_body, mesh=mesh, in_specs=in_specs,
                                out_specs=out_specs, check_rep=False),
                      keep_unused=True)
    concat_in = [
        _np.concatenate([_np.asarray(in_maps[c][nm]) for c in range(n_cores)],
                        axis=0)
        for nm in in_names
    ]
    concat_zeros = [_np.zeros((n_cores * z.shape[0], *z.shape[1:]), z.dtype)
                    for z in zero_outs]
    import time as _time
    from jax.sharding import NamedSharding
    sh = NamedSharding(mesh, PartitionSpec("core"))
    dev_in = [jax.device_put(a, sh) for a in concat_in]
    dev_zeros = [jax.device_put(a, sh) for a in concat_zeros]
    jax.block_until_ready(dev_in + dev_zeros)
    out_arrs = sharded(*dev_in, *dev_zeros)
    jax.block_until_ready(out_arrs)
    times = []
    for _ in range(reps):
        t0 = _time.perf_counter()
        out_arrs = sharded(*dev_in, *dev_zeros)
        jax.block_until_ready(out_arrs)
        times.append(_time.perf_counter() - t0)
    dt = min(times)
    results = [
        {nm: _np.asarray(out_arrs[i]).reshape(n_cores, *out_avals[i].shape)[c]
         for i, nm in enumerate(out_names)}
        for c in range(n_cores)
    ]
    return results, dt


def _bass_exec_bind(b2j, operands, out_avals, in_names, out_names, nc):
    return b2j._bass_exec_p.bind(
        *operands,
        out_avals=tuple(out_avals),
        in_names=tuple(in_names),
        out_names=tuple(out_names),
        lowering_input_output_aliases=(),
        sim_require_finite=True,
        sim_require_nnan=True,
        nc=nc,
    )


# ----------------------------------------------------------------------------
# Entry point
# ----------------------------------------------------------------------------

_CACHE = {}


def _run(inputs, trace=False):
    x = np.asarray(inputs["x"], np.float32)
    edge_index = np.asarray(inputs["edge_index"], np.int32)
    W1 = np.asarray(inputs["W1"], np.float32)
    a1s = np.asarray(inputs["att1_src"], np.float32)
    a1d = np.asarray(inputs["att1_dst"], np.float32)
    W2 = np.asarray(inputs["W2"], np.float32)
    a2s = np.asarray(inputs["att2_src"], np.float32)
    a2d = np.asarray(inputs["att2_dst"], np.float32)
    b1 = np.asarray(inputs["b1"], np.float32)
    b2 = np.asarray(inputs["b2"], np.float32)
    assert not b1.any() and not b2.any(), "nonzero bias unsupported"

    key = hashlib.sha1(
        b"v3" + edge_index.tobytes() + np.int64(x.shape).tobytes()
    ).hexdigest()
    cfg, shared, per_core = _host_prep(x, edge_index, W1, a1s, a1d, W2, a2s, a2d)
    if key not in _CACHE:
        _CACHE[key] = _build_program(cfg)
    nc = _CACHE[key]

    in_maps = []
    for k in range(cfg["n_cores"]):
        m = dict(shared)
        m.update(per_core[k])
        in_maps.append(m)
    res = run_bass_kernel_spmd(nc, in_maps, list(range(cfg["n_cores"])),
                               trace=trace)
    out = gather_out([res.results[k]["out2"] for k in range(cfg["n_cores"])],
                     cfg)
    return out.astype(np.float32), res


def gather_out(outs, cfg):
    allrows = np.concatenate(outs, axis=0)          # [R, NCLS] permuted
    return allrows[cfg["pos"][: cfg["N"]]]


def kernel(**inputs):
    out, _ = _run(inputs, trace=False)
    return out
